# revision 35
# baseline (speedup 1.0000x reference)
"""Multi-head attention (B=4, S=2048, E=768, H=12, D=64, causal) on 8 trn2
NeuronCores.

Sharding: core c -> batch b = c//2, head-half g = c%2 (6 heads each).
Each core computes its 6 heads' attention plus the partial output
projection; the host sums the two half-head partials per batch.

Hybrid precision (validated against the fp32 reference in numpy):
  - Rows q < 512 (q-window 0) have concentrated softmax (few keys), so
    their whole attention path stays bf16: bf16 qk-projection (only the
    first 512 q/k columns are needed causally), bf16 scores, bf16 exp'd
    weights, bf16 ctx with bf16 V.
  - Rows q >= 512 average fp8 quantization noise over many keys: fp8e4
    q/k (from an fp8 DoubleRow qk-projection over e-pairs), plain-fp8
    score matmuls, exp'd weights written fp8e4 by ACT (bias -2.5 keeps
    them in fp8 range; softmax normalization cancels the shift), and the
    ctx matmul runs fp8 DoubleRow over k-chunk pairs (half the
    instructions, quarter the PE cycles of bf16).
  Combined max-rel-err ~8e-3 vs the 2e-2 gate.

On-device strategy (per core) otherwise matches the bf16 design:
  - Host pre-transposes x / weight slices so every contraction dim lands
    on SBUF partitions; x ships in bf16 and fp8.
  - Scores are computed TRANSPOSED (S^T[k, q] = K^T Q); diagonal tiles
    shrink their moving window; in-diagonal-block causal masking is a
    DVE multiply by a 0/1 lower-triangle.
  - V_aug packs a ones column per head so the PE emits softmax row-sums
    for free; V is stored bf16 (k-chunks 0-3, for the bf16 window) and
    fp8 in per-unit pair layout [128, 2, 65*HPC] for DoubleRow.
  - ctx lands [q, d+1] with q on partitions; normalization is a DVE
    reciprocal + per-qc scalar multiply; ONE merged [128,128] PE
    transpose per (hp, qw, qc) brings both heads' ctx^T back.
  - The attention phase is ACT(exp)-throughput-bound; projections and
    ctx/norm/out-proj run as deferred tasks interleaved between units.
"""
import sys, json, os

for _p in ("/opt/trn_rl_repo",):
    if _p not in sys.path and os.path.isdir(_p):
        sys.path.insert(0, _p)

import numpy as np
import concourse.bass as bass
import concourse.mybir as mybir
import concourse.tile as tile
from concourse.bass_utils import run_bass_kernel_spmd

B, S, E, H, D = 4, 2048, 768, 12, 64
HPC = H // 2          # heads per core = 6
FPC = HPC * D         # features per core per q/k/v = 384
EC = E // 128         # 6 contraction chunks for projections
SC = S // 128         # 16 s-chunks
QW = S // 512         # 4 q-windows
KC = S // 128         # 16 k-chunks
F32 = mybir.dt.float32
BF16 = mybir.dt.bfloat16
FP8 = mybir.dt.float8e4
EXP = mybir.ActivationFunctionType.Exp
DR = mybir.MatmulPerfMode.DoubleRow
EBIAS = -2.5          # exp bias for fp8 windows (softmax-invariant)


def _patch_multiwait(nc, max_waits=1):
    """This container's walrus rejects instructions with more than one sync
    wait. Split excess waits onto same-engine NOPs emitted immediately
    before the instruction (same-engine streams are order-preserving)."""
    raw = nc.to_json_bytes()
    m = json.loads(raw)
    for f in m["functions"]:
        for b in f["blocks"]:
            out = []
            for inst in b["instructions"]:
                si = inst.get("sync_info") or {}
                ws = si.get("on_wait") or []
                if len(ws) > max_waits:
                    eng = inst["engine"]
                    for i, w in enumerate(ws[:-max_waits]):
                        out.append({
                            "debug": inst.get("debug", 0), "engine": eng,
                            "ins": [], "name": inst["name"] + f"-mw{i}",
                            "opcode": "NoOp", "outs": [],
                            "sync_info": {"on_update": [], "on_wait": [w]},
                        })
                    si["on_wait"] = ws[-max_waits:]
                out.append(inst)
            b["instructions"] = out
    patched = json.dumps(m).encode()
    nc.to_json_bytes = lambda: patched
    return nc


def build_nc(with_bias=True):
    nc = bass.Bass()
    xT = nc.dram_tensor("xT", [E, S], BF16, kind="ExternalInput")
    xT8 = nc.dram_tensor("xT8", [E, S], FP8, kind="ExternalInput")
    wqkT = nc.dram_tensor("wqkT", [E, 2 * FPC], BF16, kind="ExternalInput")
    wqkT8 = nc.dram_tensor("wqkT8", [E, 2 * FPC], FP8, kind="ExternalInput")
    wvT = nc.dram_tensor("wvT", [E, FPC], BF16, kind="ExternalInput")
    woT = nc.dram_tensor("woT", [FPC, E], BF16, kind="ExternalInput")
    bqk = nc.dram_tensor("bqk", [128, 2 * FPC // 128], F32, kind="ExternalInput")
    bv = nc.dram_tensor("bv", [1, FPC], BF16, kind="ExternalInput")
    bo = nc.dram_tensor("bo", [1, E], BF16, kind="ExternalInput")
    tri = nc.dram_tensor("tri", [128, 128], BF16, kind="ExternalInput")
    tri8 = nc.dram_tensor("tri8", [128, 128], FP8, kind="ExternalInput")
    ident = nc.dram_tensor("ident", [128, 128], BF16, kind="ExternalInput")
    ones = nc.dram_tensor("ones", [1, 128], BF16, kind="ExternalInput")
    y = nc.dram_tensor("y", [S, E], F32, kind="ExternalOutput")

    with tile.TileContext(nc) as tc, \
         nc.allow_low_precision(reason="hybrid bf16/fp8 pipeline by design"):
        with tc.tile_pool(name="persist", bufs=1) as P, \
             tc.tile_pool(name="ps", bufs=1, space="PSUM") as PS:
            # --- persistent tiles (bottom-of-stack, live whole kernel)
            # bf16 q/k: only columns [0, 512) are ever read (q-window 0)
            qkT_sb = [P.tile([128, 512], BF16, name=f"qkT{i}") for i in range(6)]
            # fp8 q/k: q chunks hold cols [512, 2048), k chunks [0, 2048)
            qkT8_sb = [P.tile([128, S], FP8, name=f"qkT8_{i}") for i in range(6)]
            # bf16 V for the bf16 window's ctx (k-chunks 0..3 only)
            V_sb = [P.tile([128, 65 * HPC], BF16, name=f"V{i}") for i in range(4)]
            # fp8 V in unit-pair layout: per partition [t(2), h(6), 65]
            V8_sb = [P.tile([128, 2 * 65 * HPC], FP8, name=f"V8_{i}")
                     for i in range(KC // 2)]
            ctxT_sb = [P.tile([128, S], BF16, name=f"ctxT{i}") for i in range(3)]
            woT_sb = [P.tile([128, E], BF16, name=f"woT{i}") for i in range(3)]
            bqk_sb = P.tile([128, 6], F32, name="bqk_sb")
            bv_sb = P.tile([1, FPC], BF16, name="bv_sb")
            bo_sb = P.tile([1, E], BF16, name="bo_sb")
            tri_sb = P.tile([128, 128], BF16, name="tri_sb")
            tri8_sb = P.tile([128, 128], FP8, name="tri8_sb")
            id_sb = P.tile([128, 128], BF16, name="id_sb")
            on_sb = P.tile([1, 128], BF16, name="on_sb")
            ebias_sb = P.tile([128, 1], F32, name="ebias_sb")
            nc.gpsimd.memset(ebias_sb[:], EBIAS)

            def ps_tile(shape, tag, bufs, dtype=F32):
                return PS.tile(shape, dtype, name=tag, tag=tag, bufs=bufs)

            # ============ phase 1 (projections) + attention, interleaved ====
            with tc.tile_pool(name="inp", bufs=1) as PI, \
                 tc.tile_pool(name="esb", bufs=14) as EP, \
                 tc.tile_pool(name="nrm", bufs=12) as NP, \
                 tc.tile_pool(name="osb", bufs=3) as OP:
                # consolidated input tiles; one DMA dispatch per tensor region
                xT_sb = PI.tile([128, EC * S], BF16, name="xT_all")
                xT8_sb = PI.tile([128, EC * S], FP8, name="xT8_all")
                wqkT_sb = PI.tile([128, EC * 2 * FPC], BF16, name="wqkT_all")
                wqkT8_sb = PI.tile([128, EC * 2 * FPC], FP8, name="wqkT8_all")
                wvT_sb = PI.tile([128, EC * FPC], BF16, name="wvT_all")
                xs = xT_sb[:].rearrange("p (e s) -> p e s", e=EC)
                xd = xT.ap().rearrange("(e p) s -> p e s", p=128)
                x8s = xT8_sb[:].rearrange("p (e s) -> p e s", e=EC)
                x8d = xT8.ap().rearrange("(e p) s -> p e s", p=128)
                qs = wqkT_sb[:].rearrange("p (e f) -> p e f", e=EC)
                qd = wqkT.ap().rearrange("(e p) f -> p e f", p=128)
                q8s = wqkT8_sb[:].rearrange("p (e f) -> p e f", e=EC)
                q8d = wqkT8.ap().rearrange("(e p) f -> p e f", p=128)
                # DMA order: startup chains first (bf16 wqkT fo 0/3, then xT
                # cols 0:512 per e-chunk so the startup chains pipeline with
                # the DMA stream), then the rest in rough order of first use.
                nc.sync.dma_start(qs[:, :, 0:128], qd[:, :, 0:128])
                nc.sync.dma_start(qs[:, :, 384:512], qd[:, :, 384:512])
                nc.sync.dma_start(xs[:, 0:3, 0:512], xd[:, 0:3, 0:512])
                nc.sync.dma_start(xs[:, 3:6, 0:512], xd[:, 3:6, 0:512])
                nc.sync.dma_start(tri_sb[:], tri.ap())
                nc.sync.dma_start(q8s[:], q8d[:])
                nc.sync.dma_start(x8s[:, :, 0:1024], x8d[:, :, 0:1024])
                nc.sync.dma_start(
                    wvT_sb[:].rearrange("p (e f) -> p e f", e=EC),
                    wvT.ap().rearrange("(e p) f -> p e f", p=128))
                nc.sync.dma_start(tri8_sb[:], tri8.ap())
                nc.sync.dma_start(xs[:, :, 512:S], xd[:, :, 512:S])
                nc.sync.dma_start(x8s[:, :, 1024:S], x8d[:, :, 1024:S])
                nc.sync.dma_start(qs[:, :, 128:384], qd[:, :, 128:384])
                nc.sync.dma_start(qs[:, :, 512:768], qd[:, :, 512:768])
                nc.sync.dma_start(id_sb[:], ident.ap())
                for i in range(3):
                    nc.sync.dma_start(woT_sb[i][:],
                                      woT.ap()[128 * i:128 * (i + 1), :])
                nc.sync.dma_start(bqk_sb[:], bqk.ap())
                nc.sync.dma_start(bv_sb[:], bv.ap())
                nc.sync.dma_start(on_sb[:], ones.ap())
                nc.sync.dma_start(bo_sb[:], bo.ap())

                def emit_qk_chain(fo, on_act=False, cols=(0, 512)):
                    """bf16 qk-proj for f-chunk fo, s-cols [0, 512) only
                    (optionally a sub-range, for the startup k-chain)."""
                    c0, c1 = cols
                    pair = ps_tile([128, 512], "ppro_t", 2)
                    for ecc in range(EC):
                        nc.tensor.matmul(
                            pair[:, c0:c1],
                            wqkT_sb[:, 768 * ecc + 128 * fo:
                                    768 * ecc + 128 * (fo + 1)],
                            xT_sb[:, S * ecc + c0:S * ecc + c1],
                            start=(ecc == 0), stop=(ecc == EC - 1),
                            skip_group_check=True)
                    dst = qkT_sb[fo][:, c0:c1]
                    src = pair[:, c0:c1]
                    if with_bias:
                        nc.vector.tensor_scalar_add(
                            dst, src, bqk_sb[:, fo:fo + 1])
                    elif on_act:
                        nc.scalar.copy(dst, src)
                    else:
                        nc.vector.tensor_copy(dst, src)

                def emit_qk8_chain(fo, win):
                    """fp8 DoubleRow qk-proj for f-chunk fo, one 256-col
                    window. q chunks (fo<3): win 0..5 -> cols 512+256*win.
                    k chunks (fo>=3): win 0..7 -> cols 256*win."""
                    c0 = (512 if fo < 3 else 0) + 256 * win
                    pair = ps_tile([128, 512], "ppro_t", 2)
                    wv = wqkT8_sb[:].rearrange("p (e f) -> p e f", e=EC)
                    xv = xT8_sb[:].rearrange("p (e s) -> p e s", e=EC)
                    for ecp in range(EC // 2):
                        nc.tensor.matmul(
                            pair[:, 0:256],
                            wv[:, 2 * ecp:2 * ecp + 2,
                               128 * fo:128 * (fo + 1)],
                            xv[:, 2 * ecp:2 * ecp + 2, c0:c0 + 256],
                            start=(ecp == 0), stop=(ecp == EC // 2 - 1),
                            skip_group_check=True, perf_mode=DR)
                    dst = qkT8_sb[fo][:, c0:c0 + 256]
                    src = pair[:, 0:256]
                    if with_bias:
                        nc.vector.tensor_scalar_add(
                            dst, src, bqk_sb[:, fo:fo + 1])
                    else:
                        nc.vector.tensor_copy(dst, src)

                def emit_v_chunk(sc):
                    """V-proj for s-chunk sc (bf16), copied out bf16 (sc<4,
                    for the bf16 window) and fp8 (unit-pair layout, always)."""
                    psv = ps_tile([128, FPC], "ppro_t", 2)
                    for ecc in range(EC):
                        nc.tensor.matmul(
                            psv[:],
                            xT_sb[:, S * ecc + 128 * sc:
                                  S * ecc + 128 * (sc + 1)],
                            wvT_sb[:, FPC * ecc:FPC * (ecc + 1)],
                            start=(ecc == 0),
                            stop=(not with_bias and ecc == EC - 1),
                            skip_group_check=True)
                    if with_bias:
                        nc.tensor.matmul(psv[:], on_sb[:, 0:128],
                                         bv_sb[:], start=False, stop=True,
                                         skip_group_check=True)
                    u, t = divmod(sc, 2)
                    v8 = V8_sb[u][:].rearrange("p (t h x) -> p t h x",
                                               t=2, x=65)
                    nc.vector.tensor_copy(
                        v8[:, t, :, 0:64],
                        psv[:].rearrange("p (h x) -> p h x", x=64))
                    nc.gpsimd.memset(v8[:, t, :, 64:65], 1.0)
                    if sc < 4:
                        vv = V_sb[sc][:].rearrange("p (h x) -> p h x", x=65)
                        nc.vector.tensor_copy(
                            vv[:, :, 0:64],
                            psv[:].rearrange("p (h x) -> p h x", x=64))
                        nc.gpsimd.memset(vv[:, :, 64:65], 1.0)

                def emit_scores(hp, qw, u):
                    """Scores S^T[k, q] for a pair of k-chunks, both heads,
                    + exp (+ DVE causal masks on diag blocks). qw=0 runs
                    bf16 (writes Et bf16); qw>=1 runs fp8 (writes Et fp8,
                    exp bias EBIAS). Returns Et [128, 2048]
                    (cols 1024*hd + 512*half + qlocal)."""
                    fp8 = qw >= 1
                    if fp8:
                        qT, kT = qkT8_sb[hp], qkT8_sb[3 + hp]
                        qcol = 512 * qw
                        trit = tri8_sb
                        edt = FP8
                    else:
                        qT, kT = qkT_sb[hp], qkT_sb[3 + hp]
                        qcol = 0       # bf16 tiles hold only cols [0,512)
                        trit = tri_sb
                        edt = BF16
                    pss = {hd: ps_tile([128, 1024], "pss_t", 2)
                           for hd in range(2)}
                    Et = EP.tile([128, 2048], edt, name="E_t")
                    for half in range(2):
                        ki = 2 * u + half
                        j = ki - 4 * qw
                        c = 128 * j if j > 0 else 0
                        # strict row-group alternation (base 0,64,0,64) so
                        # score matmul pairs run on separate PE row groups;
                        # diag tiles shrink the moving window
                        for hd in range(2):
                            base = 64 * hd
                            nc.tensor.matmul(
                                pss[hd][:, 512 * half + c:512 * (half + 1)],
                                kT[base:base + 64, 128 * ki:128 * (ki + 1)],
                                qT[base:base + 64,
                                   qcol + c:qcol + 512],
                                start=True, stop=True,
                                skip_group_check=True)
                    j0 = 2 * u - 4 * qw
                    j1 = j0 + 1
                    c0 = 128 * j0 if j0 > 0 else 0
                    bias = ebias_sb[:, 0:1] if fp8 else 0.0
                    # one exp per head spans both halves when contiguous;
                    # when the half-1 diag shrink leaves an unwritten PSUM
                    # gap, split the exp around it
                    for hd in range(2):
                        if j1 > 0:
                            nc.scalar.activation(
                                Et[:, 1024 * hd + c0:1024 * hd + 512],
                                pss[hd][:, c0:512], EXP, scale=0.125,
                                bias=bias)
                            c1 = 128 * j1
                            nc.scalar.activation(
                                Et[:, 1024 * hd + 512 + c1:1024 * (hd + 1)],
                                pss[hd][:, 512 + c1:1024], EXP, scale=0.125,
                                bias=bias)
                        else:
                            nc.scalar.activation(
                                Et[:, 1024 * hd + c0:1024 * (hd + 1)],
                                pss[hd][:, c0:1024], EXP, scale=0.125,
                                bias=bias)
                    # causal mask inside the diagonal 128x128 blocks
                    for half in range(2):
                        j = 2 * u + half - 4 * qw
                        if j >= 0:
                            for hd in range(2):
                                off = 1024 * hd + 512 * half + 128 * j
                                nc.vector.tensor_mul(
                                    Et[:, off:off + 128],
                                    Et[:, off:off + 128], trit[:])
                    return Et

                def emit_ctx_qc(hp, qw, qc, Ets, psc):
                    """ctx for one q-chunk, both heads. qw=0: bf16 per-ki
                    matmuls with V_sb. qw>=1: fp8 DoubleRow over k-chunk
                    pairs (units) with V8_sb, plus a plain-fp8 tail when
                    the diagonal cuts a unit in half."""
                    nk = 4 * qw + qc + 1        # k-chunks 0..4qw+qc
                    if qw == 0:
                        for ki in range(nk):
                            u, half = divmod(ki, 2)
                            Et = Ets[u]
                            for hd in range(2):
                                h = 2 * hp + hd
                                nc.tensor.matmul(
                                    psc[hd][:, 65 * qc:65 * qc + 65],
                                    Et[:, 1024 * hd + 512 * half + 128 * qc:
                                        1024 * hd + 512 * half + 128 * qc
                                        + 128],
                                    V_sb[ki][:, 65 * h:65 * h + 65],
                                    start=(ki == 0), stop=(ki == nk - 1),
                                    skip_group_check=True)
                        return
                    nu_full = nk // 2           # full DR unit-pairs
                    tail = nk % 2               # lone half-0 chunk at the end
                    for hd in range(2):
                        h = 2 * hp + hd
                        for u in range(nu_full):
                            ev = Ets[u][:, 1024 * hd:1024 * (hd + 1)] \
                                .rearrange("p (t q) -> p t q", t=2)
                            v8 = V8_sb[u][:].rearrange(
                                "p (t h x) -> p t h x", t=2, x=65)
                            nc.tensor.matmul(
                                psc[hd][:, 65 * qc:65 * qc + 65],
                                ev[:, :, 128 * qc:128 * qc + 128],
                                v8[:, :, h, :],
                                start=(u == 0),
                                stop=(tail == 0 and u == nu_full - 1),
                                skip_group_check=True, perf_mode=DR)
                        if tail:
                            u = nu_full
                            v8 = V8_sb[u][:].rearrange(
                                "p (t h x) -> p t h x", t=2, x=65)
                            nc.tensor.matmul(
                                psc[hd][:, 65 * qc:65 * qc + 65],
                                Ets[u][:, 1024 * hd + 128 * qc:
                                       1024 * hd + 128 * qc + 128],
                                v8[:, 0, h, :],
                                start=(nu_full == 0), stop=True,
                                skip_group_check=True)

                def emit_finish_qc(hp, qw, qc, psc, st, outproj=True):
                    """Per-qc finish for the final step: craw slice + recip +
                    normalize + transpose + ctxT copy + out-proj, so each
                    s-chunk's chain starts the moment its own ctx is done."""
                    if "ctxn" not in st:
                        st["ctxn"] = NP.tile([128, 512], BF16, name="ctxn_t")
                    # psc_t's 2 slots are BOTH live (the psc accumulators)
                    # until the last craw is copied out, so pt must come from
                    # the pss_t ring (scores are done with it by now).
                    pt = ps_tile([128, 512], "pss_t", 2, dtype=BF16)
                    ctxn = st["ctxn"]
                    for hd in range(2):
                        craw = NP.tile([128, 65], F32, name="crawq_t")
                        nc.vector.tensor_copy(
                            craw[:], psc[hd][:, 65 * qc:65 * qc + 65])
                        rinv = NP.tile([128, 1], F32, name="rinvq_t")
                        nc.vector.reciprocal(
                            rinv[:], craw[:, 64:65])
                        nc.vector.tensor_scalar_mul(
                            ctxn[:, 128 * qc + 64 * hd:
                                 128 * qc + 64 * (hd + 1)],
                            craw[:, 0:64], rinv[:])
                    nc.tensor.transpose(
                        pt[:, 128 * qc:128 * (qc + 1)],
                        ctxn[:, 128 * qc:128 * (qc + 1)], id_sb[:])
                    nc.vector.tensor_copy(
                        ctxT_sb[hp][:, 512 * qw + 128 * qc:
                                     512 * qw + 128 * (qc + 1)],
                        pt[:, 128 * qc:128 * (qc + 1)])
                    if outproj:
                        emit_outproj_sc(4 * qw + qc, tail=True)

                def emit_norm_head(hp, qw, psc, st):
                    """Copy raw ctx out of PSUM (freeing psc) and compute
                    reciprocal row-sums."""
                    craws = []
                    for hd in range(2):
                        craw = NP.tile([128, 260], F32, name="craw_t")
                        nc.vector.tensor_copy(craw[:], psc[hd][:])
                        craws.append(craw)
                    pt = ps_tile([128, 512], "psc_t", 2, dtype=BF16)
                    ctxn = NP.tile([128, 512], BF16, name="ctxn_t")
                    rinvs = []
                    for hd in range(2):
                        craw = craws[hd]
                        cv = craw[:].rearrange("p (q x) -> p q x", x=65)
                        rinv = NP.tile([128, 4], F32, name="rinv_t")
                        nc.vector.reciprocal(
                            rinv[:].rearrange("p (q x) -> p q x", x=1),
                            cv[:, :, 64:65])
                        rinvs.append((craw, rinv))
                    st["pt"] = pt
                    st["ctxn"] = ctxn
                    st["rinvs"] = rinvs

                def emit_norm_qc(hp, qw, qc, st):
                    """Normalize + ONE merged 2-head transpose + copy out one
                    128-column ctxT block."""
                    pt, ctxn, rinvs = st["pt"], st["ctxn"], st["rinvs"]
                    for hd in range(2):
                        craw, rinv = rinvs[hd]
                        nc.vector.tensor_scalar_mul(
                            ctxn[:, 128 * qc + 64 * hd:
                                 128 * qc + 64 * (hd + 1)],
                            craw[:, 65 * qc:65 * qc + 64],
                            rinv[:, qc:qc + 1])
                    nc.tensor.transpose(
                        pt[:, 128 * qc:128 * (qc + 1)],
                        ctxn[:, 128 * qc:128 * (qc + 1)], id_sb[:])
                    nc.vector.tensor_copy(
                        ctxT_sb[hp][:, 512 * qw + 128 * qc:
                                     512 * qw + 128 * (qc + 1)],
                        pt[:, 128 * qc:128 * (qc + 1)])

                def emit_outproj_sc(sc, tail=False, ring=None):
                    osb = OP.tile([128, E], F32, name="osb_t")
                    # tail chains run while psc_t's two slots are still live
                    # (the psc accumulators), so they must use pss_t;
                    # explicit ring= alternates banks for back-to-back chains
                    tg, nb = (ring, 2) if ring else \
                        (("pss_t", 2) if tail else ("psc_t", 2))
                    pos = {0: ps_tile([128, 512], tg, nb),
                           512: ps_tile([128, 256], tg, nb)}
                    for c in range(3):
                        for f0, fn in ((0, 512), (512, 256)):
                            nc.tensor.matmul(
                                pos[f0][:, 0:fn],
                                ctxT_sb[c][:, 128 * sc:128 * (sc + 1)],
                                woT_sb[c][:, f0:f0 + fn],
                                start=(c == 0),
                                stop=(not with_bias and c == 2),
                                skip_group_check=True)
                    for f0, fn in ((0, 512), (512, 256)):
                        if with_bias:
                            nc.tensor.matmul(pos[f0][:, 0:fn],
                                             on_sb[:, 0:128],
                                             bo_sb[:, f0:f0 + fn],
                                             start=False, stop=True,
                                             skip_group_check=True)
                        nc.vector.tensor_copy(osb[:, f0:f0 + fn],
                                              pos[f0][:, 0:fn])
                    nc.sync.dma_start(y.ap()[128 * sc:128 * (sc + 1), :],
                                      osb[:])

                def emit_attention():
                    # software pipeline: ctx runs as per-q-chunk deferred
                    # tasks queued when a step's scores complete; one task
                    # pops per unit so ctx/norm/out-proj spread between
                    # later units while ACT chews on exps.
                    work = []

                    def flush_one():
                        if work:
                            work.pop(0)()
                        if len(work) > 3:
                            work.pop(0)()

                    def make_step(hp, qw, Ets):
                        holder = {}

                        def get_psc():
                            if not holder:
                                holder[0] = {
                                    hd: ps_tile([128, 260], "psc_t", 2)
                                    for hd in range(2)}
                            return holder[0]

                        def ctx_task(qc):
                            return lambda: emit_ctx_qc(
                                hp, qw, qc, Ets, get_psc())
                        return get_psc, ctx_task

                    def finish_step(hp, qw, get_psc):
                        st = {}
                        if hp < 2:
                            def norm_all():
                                emit_norm_head(hp, qw, get_psc(), st)
                                for qc in range(4):
                                    emit_norm_qc(hp, qw, qc, st)
                            work.append(norm_all)
                        else:
                            work.append(lambda: emit_norm_head(
                                hp, qw, get_psc(), st))

                            def norm_op(qc):
                                emit_norm_qc(hp, qw, qc, st)
                                emit_outproj_sc(4 * qw + qc)
                            for qc in range(4):
                                work.append(lambda qc=qc: norm_op(qc))

                    # phase-1 chains interleaved between attention units:
                    # (hp, qw, u) -> thunks emitted right after that unit's
                    # scores+flush. Deadlines:
                    #   bf16 qk chain fo in {hp', 3+hp'} before (hp', 0, 0)
                    #   fp8 k chain (3+hp', win w) before (hp', 1 + w//4,
                    #     u = w % ...) - k-chunks 2w,2w+1 first read at
                    #     (hp', qw >= 1, u = w)
                    #   fp8 q chain (hp', win w) before (hp', 1 + w//2, 0)
                    #   V chunk sc: read by ctx tasks popping ~one step later
                    intra = {}

                    def add(hp, qw, u, fn):
                        intra.setdefault((hp, qw, u), []).append(fn)

                    def addv(hp, qw, u, sc):
                        add(hp, qw, u, lambda: emit_v_chunk(sc))

                    def addqk(hp, qw, u, fo):
                        add(hp, qw, u, lambda: emit_qk_chain(fo))

                    def addq8(hp, qw, u, fo, w):
                        add(hp, qw, u, lambda: emit_qk8_chain(fo, w))

                    # --- hp 0 --- (all 16 V chunks live here; k8(3,*) and
                    # q8(0,*) feed hp0's own fp8 windows)
                    # Step order: plain nested (hp, qw); (2,3) runs
                    # last with the inline early-ctx finish.
                    steps = [(hp, qw) for hp in range(3)
                             for qw in range(QW)]
                    # Placement rule: an intra item at slot (hp, qw, u) is
                    # emitted AFTER scores of unit u+1 (emit-ahead pipeline),
                    # so it may only feed units >= u+2 of its own step.
                    addq8(0, 0, 0, 3, 0)
                    addq8(0, 0, 0, 0, 0)
                    addv(0, 0, 0, 0)
                    addq8(0, 0, 1, 3, 1)
                    addq8(0, 0, 1, 0, 1)
                    addv(0, 0, 1, 1)
                    addv(0, 1, 0, 2)
                    addq8(0, 1, 0, 3, 2)
                    addv(0, 1, 1, 3)
                    addq8(0, 1, 1, 3, 3)
                    addq8(0, 1, 1, 0, 2)
                    addq8(0, 1, 2, 0, 3)
                    addv(0, 1, 3, 4)
                    addq8(0, 1, 3, 3, 4)
                    addv(0, 2, 0, 5)
                    addq8(0, 2, 0, 3, 5)
                    addv(0, 2, 1, 6)
                    addq8(0, 2, 1, 0, 4)
                    addv(0, 2, 2, 7)
                    addq8(0, 2, 3, 0, 5)
                    addv(0, 2, 3, 8)
                    addq8(0, 2, 4, 3, 6)
                    addv(0, 2, 4, 9)
                    addq8(0, 2, 5, 3, 7)
                    addv(0, 3, 0, 10)
                    addv(0, 3, 1, 11)
                    addv(0, 3, 2, 12)
                    addv(0, 3, 3, 13)
                    addv(0, 3, 4, 14)
                    addqk(0, 3, 5, 1)
                    addv(0, 3, 5, 15)
                    addqk(0, 3, 6, 4)
                    addq8(0, 3, 6, 4, 0)
                    addq8(0, 3, 7, 1, 0)
                    addq8(1, 0, 0, 4, 1)
                    addq8(1, 0, 0, 1, 1)
                    addq8(1, 0, 1, 4, 2)
                    addq8(1, 1, 0, 4, 3)
                    addq8(1, 1, 1, 1, 2)
                    addq8(1, 1, 2, 1, 3)
                    addq8(1, 1, 3, 4, 4)
                    addq8(1, 2, 0, 4, 5)
                    addq8(1, 2, 1, 1, 4)
                    addq8(1, 2, 3, 1, 5)
                    addq8(1, 2, 4, 4, 6)
                    addq8(1, 2, 5, 4, 7)
                    addqk(1, 3, 1, 2)
                    addqk(1, 3, 2, 5)
                    addq8(1, 3, 3, 5, 0)
                    addq8(1, 3, 4, 5, 1)
                    addq8(1, 3, 5, 2, 0)
                    addq8(1, 3, 6, 2, 1)
                    addq8(1, 3, 7, 5, 2)
                    addq8(2, 0, 0, 5, 3)
                    addq8(2, 0, 1, 2, 2)
                    addq8(2, 1, 0, 5, 4)
                    addq8(2, 1, 1, 2, 3)
                    addq8(2, 1, 2, 5, 5)
                    addq8(2, 2, 0, 5, 6)
                    addq8(2, 2, 1, 2, 4)
                    addq8(2, 2, 3, 2, 5)
                    addq8(2, 2, 4, 5, 7)

                    for si, (hp, qw) in enumerate(steps):
                        nu = 2 * qw + 2
                        last = (si == len(steps) - 1)
                        Ets = []
                        get_psc, ctx_task = make_step(hp, qw, Ets)
                        if last:
                            # run all but the final two units normally (with
                            # harder draining), then emit the last two units'
                            # scores up front so ACT stays fed while the PE
                            # drains the backlog and runs the tail chains
                            Ets.append(emit_scores(hp, qw, 0))
                            for u in range(nu - 2):
                                if u + 1 < nu - 2:
                                    Ets.append(emit_scores(hp, qw, u + 1))
                                for fn in intra.get((hp, qw, u), ()):
                                    fn()
                                flush_one()
                                flush_one()
                            Ets.append(emit_scores(hp, qw, nu - 2))
                            Ets.append(emit_scores(hp, qw, nu - 1))
                            while work:
                                work.pop(0)()
                            st = {}
                            for qc in range(4):
                                ctx_task(qc)()
                            for qc in range(4):
                                emit_finish_qc(hp, qw, qc, get_psc(), st,
                                               outproj=False)
                            for qc in range(4):
                                emit_outproj_sc(4 * qw + qc, tail=True)
                            continue
                        # emit-ahead software pipeline                        # emit-ahead software pipeline: unit u+1's scores go
                        # out BEFORE unit u's intra chains / deferred pops, so
                        # the next exps are never queued behind filler work on
                        # the in-order PE stream.
                        Ets.append(emit_scores(hp, qw, 0))
                        for u in range(nu):
                            if u + 1 < nu:
                                Ets.append(emit_scores(hp, qw, u + 1))
                            for fn in intra.get((hp, qw, u), ()):
                                fn()
                            flush_one()
                        for qc in range(4):
                            work.append(ctx_task(qc))
                        finish_step(hp, qw, get_psc)
                    while work:
                        flush_one()

                # PE p-state warmup: dummy matmuls on a memset tile keep the
                # PE busy through the startup DMAs so the real startup chains
                # run at a higher clock (the cost model ramps 0.65 -> 1.2 ->
                # 2.4 GHz with continuous execution).
                warm_sb = PI.tile([128, 128], BF16, name="warm_sb")
                nc.gpsimd.memset(warm_sb[:], 0.5)
                wps = ps_tile([128, 512], "ppro_t", 2)
                for _ in range(24):
                    nc.tensor.matmul(wps[:, 0:128], warm_sb[:], warm_sb[:],
                                     start=True, stop=True,
                                     skip_group_check=True)

                # start-up: only what the first scores need
                emit_qk_chain(0)
                emit_qk_chain(3, on_act=True)
                emit_attention()

    return _patch_multiwait(nc)


_NC = {}


def _get_nc(with_bias=True):
    if with_bias not in _NC:
        _NC[with_bias] = build_nc(with_bias=with_bias)
    return _NC[with_bias]


def _prep_core_inputs(x, in_proj_w, in_proj_b, out_w, out_b):
    """Build the 8 per-core input dicts (host-side shard + transpose)."""
    import ml_dtypes
    bf16 = ml_dtypes.bfloat16
    fp8 = ml_dtypes.float8_e4m3
    # 0/1 keep-mask for S^T[k, q] diagonal blocks: keep where k <= q
    tri_np = (np.arange(128)[:, None] <= np.arange(128)[None, :])
    tri_bf = tri_np.astype(bf16)
    tri_f8 = tri_np.astype(fp8)
    id_bf = np.eye(128, dtype=np.float32).astype(bf16)
    ones_np = np.ones((1, 128), np.float32).astype(bf16)

    xT_by_b = [np.ascontiguousarray(np.asarray(x[b]).T) for b in range(B)]
    xT_bf = [a.astype(bf16) for a in xT_by_b]
    xT_f8 = [a.astype(fp8) for a in xT_by_b]

    in_maps = []
    for c in range(8):
        b = c // 2
        g = c % 2
        f0 = FPC * g
        Wq = np.asarray(in_proj_w[f0:f0 + FPC])
        Wk = np.asarray(in_proj_w[E + f0:E + f0 + FPC])
        Wv = np.asarray(in_proj_w[2 * E + f0:2 * E + f0 + FPC])
        bq = np.asarray(in_proj_b[f0:f0 + FPC])
        bk = np.asarray(in_proj_b[E + f0:E + f0 + FPC])
        bvv = np.asarray(in_proj_b[2 * E + f0:2 * E + f0 + FPC])
        Wo = np.asarray(out_w[:, f0:f0 + FPC])
        bqk_np = np.concatenate([bq, bk]).astype(np.float32).reshape(6, 128).T
        wqkT_np = np.ascontiguousarray(
            np.concatenate([Wq, Wk], axis=0).T).astype(np.float32)
        in_maps.append({
            "xT": xT_bf[b],
            "xT8": xT_f8[b],
            "wqkT": wqkT_np.astype(bf16),
            "wqkT8": wqkT_np.astype(fp8),
            "wvT": np.ascontiguousarray(Wv.T).astype(bf16),
            "woT": np.ascontiguousarray(Wo.T).astype(bf16),
            "bqk": np.ascontiguousarray(bqk_np),
            "bv": bvv.reshape(1, FPC).astype(bf16),
            # out bias only on even cores so the host-side pair-sum is exact
            "bo": np.asarray(out_b).reshape(1, E).astype(bf16) if g == 0
                  else np.zeros((1, E), bf16),
            "tri": tri_bf,
            "tri8": tri_f8,
            "ident": id_bf,
            "ones": ones_np,
        })
    return in_maps


def kernel(x, in_proj_w, in_proj_b, out_w, out_b):
    zero_bias = (not np.any(np.asarray(in_proj_b))) and \
                (not np.any(np.asarray(out_b)))
    nc = _get_nc(with_bias=not zero_bias)
    in_maps = _prep_core_inputs(x, in_proj_w, in_proj_b, out_w, out_b)
    res = run_bass_kernel_spmd(nc, in_maps, core_ids=list(range(8)))
    out = np.empty((B, S, E), np.float32)
    for b in range(B):
        out[b] = res.results[2 * b]["y"] + res.results[2 * b + 1]["y"]
    return out


# revision 37
# speedup vs baseline: 1.0217x; 1.0217x over previous
"""Multi-head attention (B=4, S=2048, E=768, H=12, D=64, causal) on 8 trn2
NeuronCores.

Sharding: core c -> batch b = c//2, head-half g = c%2 (6 heads each).
Each core computes its 6 heads' attention plus the partial output
projection; the host sums the two half-head partials per batch.

Hybrid precision (validated against the fp32 reference in numpy):
  - Rows q < 512 (q-window 0) have concentrated softmax (few keys), so
    their whole attention path stays bf16: bf16 qk-projection (only the
    first 512 q/k columns are needed causally), bf16 scores, bf16 exp'd
    weights, bf16 ctx with bf16 V.
  - Rows q >= 512 average fp8 quantization noise over many keys: fp8e4
    q/k (from an fp8 DoubleRow qk-projection over e-pairs), plain-fp8
    score matmuls, exp'd weights written fp8e4 by ACT (bias -2.5 keeps
    them in fp8 range; softmax normalization cancels the shift), and the
    ctx matmul runs fp8 DoubleRow over k-chunk pairs (half the
    instructions, quarter the PE cycles of bf16).
  Combined max-rel-err ~8e-3 vs the 2e-2 gate.

On-device strategy (per core) otherwise matches the bf16 design:
  - Host pre-transposes x / weight slices so every contraction dim lands
    on SBUF partitions; x ships in bf16 and fp8.
  - Scores are computed TRANSPOSED (S^T[k, q] = K^T Q); diagonal tiles
    shrink their moving window; in-diagonal-block causal masking is a
    DVE multiply by a 0/1 lower-triangle.
  - V_aug packs a ones column per head so the PE emits softmax row-sums
    for free; V is stored bf16 (k-chunks 0-3, for the bf16 window) and
    fp8 in per-unit pair layout [128, 2, 65*HPC] for DoubleRow.
  - ctx lands [q, d+1] with q on partitions; normalization is a DVE
    reciprocal + per-qc scalar multiply; ONE merged [128,128] PE
    transpose per (hp, qw, qc) brings both heads' ctx^T back.
  - The attention phase is ACT(exp)-throughput-bound; projections and
    ctx/norm/out-proj run as deferred tasks interleaved between units.
"""
import sys, json, os

for _p in ("/opt/trn_rl_repo",):
    if _p not in sys.path and os.path.isdir(_p):
        sys.path.insert(0, _p)

import numpy as np
import concourse.bass as bass
import concourse.mybir as mybir
import concourse.tile as tile
from concourse.bass_utils import run_bass_kernel_spmd

B, S, E, H, D = 4, 2048, 768, 12, 64
HPC = H // 2          # heads per core = 6
FPC = HPC * D         # features per core per q/k/v = 384
EC = E // 128         # 6 contraction chunks for projections
SC = S // 128         # 16 s-chunks
QW = S // 512         # 4 q-windows
KC = S // 128         # 16 k-chunks
F32 = mybir.dt.float32
BF16 = mybir.dt.bfloat16
FP8 = mybir.dt.float8e4
EXP = mybir.ActivationFunctionType.Exp
DR = mybir.MatmulPerfMode.DoubleRow
EBIAS = -2.5          # exp bias for fp8 windows (softmax-invariant)


def _patch_multiwait(nc, max_waits=1):
    """This container's walrus rejects instructions with more than one sync
    wait. Split excess waits onto same-engine NOPs emitted immediately
    before the instruction (same-engine streams are order-preserving)."""
    raw = nc.to_json_bytes()
    m = json.loads(raw)
    for f in m["functions"]:
        for b in f["blocks"]:
            out = []
            for inst in b["instructions"]:
                si = inst.get("sync_info") or {}
                ws = si.get("on_wait") or []
                if len(ws) > max_waits:
                    eng = inst["engine"]
                    for i, w in enumerate(ws[:-max_waits]):
                        out.append({
                            "debug": inst.get("debug", 0), "engine": eng,
                            "ins": [], "name": inst["name"] + f"-mw{i}",
                            "opcode": "NoOp", "outs": [],
                            "sync_info": {"on_update": [], "on_wait": [w]},
                        })
                    si["on_wait"] = ws[-max_waits:]
                out.append(inst)
            b["instructions"] = out
    patched = json.dumps(m).encode()
    nc.to_json_bytes = lambda: patched
    return nc


def build_nc(with_bias=True):
    nc = bass.Bass()
    xT = nc.dram_tensor("xT", [E, S], BF16, kind="ExternalInput")
    xT8 = nc.dram_tensor("xT8", [E, S], FP8, kind="ExternalInput")
    wqkT = nc.dram_tensor("wqkT", [E, 2 * FPC], BF16, kind="ExternalInput")
    wqkT8 = nc.dram_tensor("wqkT8", [E, 2 * FPC], FP8, kind="ExternalInput")
    wvT = nc.dram_tensor("wvT", [E, FPC], BF16, kind="ExternalInput")
    woT = nc.dram_tensor("woT", [FPC, E], BF16, kind="ExternalInput")
    bqk = nc.dram_tensor("bqk", [128, 2 * FPC // 128], F32, kind="ExternalInput")
    bv = nc.dram_tensor("bv", [1, FPC], BF16, kind="ExternalInput")
    bo = nc.dram_tensor("bo", [1, E], BF16, kind="ExternalInput")
    tri = nc.dram_tensor("tri", [128, 128], BF16, kind="ExternalInput")
    tri8 = nc.dram_tensor("tri8", [128, 128], FP8, kind="ExternalInput")
    ident = nc.dram_tensor("ident", [128, 128], BF16, kind="ExternalInput")
    ones = nc.dram_tensor("ones", [1, 128], BF16, kind="ExternalInput")
    y = nc.dram_tensor("y", [S, E], F32, kind="ExternalOutput")

    with tile.TileContext(nc) as tc, \
         nc.allow_low_precision(reason="hybrid bf16/fp8 pipeline by design"):
        with tc.tile_pool(name="persist", bufs=1) as P, \
             tc.tile_pool(name="ps", bufs=1, space="PSUM") as PS:
            # --- persistent tiles (bottom-of-stack, live whole kernel)
            # bf16 q/k: only columns [0, 512) are ever read (q-window 0)
            qkT_sb = [P.tile([128, 512], BF16, name=f"qkT{i}") for i in range(6)]
            # fp8 q/k: q chunks hold cols [512, 2048), k chunks [0, 2048)
            qkT8_sb = [P.tile([128, S], FP8, name=f"qkT8_{i}") for i in range(6)]
            # bf16 V for the bf16 window's ctx (k-chunks 0..3 only)
            V_sb = [P.tile([128, 65 * HPC], BF16, name=f"V{i}") for i in range(4)]
            # fp8 V in unit-pair layout: per partition [t(2), h(6), 65]
            V8_sb = [P.tile([128, 2 * 65 * HPC], FP8, name=f"V8_{i}")
                     for i in range(KC // 2)]
            ctxT_sb = [P.tile([128, S], BF16, name=f"ctxT{i}") for i in range(3)]
            woT_sb = [P.tile([128, E], BF16, name=f"woT{i}") for i in range(3)]
            bqk_sb = P.tile([128, 6], F32, name="bqk_sb")
            bv_sb = P.tile([1, FPC], BF16, name="bv_sb")
            bo_sb = P.tile([1, E], BF16, name="bo_sb")
            tri_sb = P.tile([128, 128], BF16, name="tri_sb")
            tri8_sb = P.tile([128, 128], FP8, name="tri8_sb")
            id_sb = P.tile([128, 128], BF16, name="id_sb")
            on_sb = P.tile([1, 128], BF16, name="on_sb")
            ebias_sb = P.tile([128, 1], F32, name="ebias_sb")
            nc.gpsimd.memset(ebias_sb[:], EBIAS)

            def ps_tile(shape, tag, bufs, dtype=F32):
                return PS.tile(shape, dtype, name=tag, tag=tag, bufs=bufs)

            # ============ phase 1 (projections) + attention, interleaved ====
            with tc.tile_pool(name="inp", bufs=1) as PI, \
                 tc.tile_pool(name="esb", bufs=14) as EP, \
                 tc.tile_pool(name="nrm", bufs=12) as NP, \
                 tc.tile_pool(name="osb", bufs=3) as OP:
                # consolidated input tiles; one DMA dispatch per tensor region
                xT_sb = PI.tile([128, EC * S], BF16, name="xT_all")
                xT8_sb = PI.tile([128, EC * S], FP8, name="xT8_all")
                wqkT_sb = PI.tile([128, EC * 2 * FPC], BF16, name="wqkT_all")
                wqkT8_sb = PI.tile([128, EC * 2 * FPC], FP8, name="wqkT8_all")
                wvT_sb = PI.tile([128, EC * FPC], BF16, name="wvT_all")
                xs = xT_sb[:].rearrange("p (e s) -> p e s", e=EC)
                xd = xT.ap().rearrange("(e p) s -> p e s", p=128)
                x8s = xT8_sb[:].rearrange("p (e s) -> p e s", e=EC)
                x8d = xT8.ap().rearrange("(e p) s -> p e s", p=128)
                qs = wqkT_sb[:].rearrange("p (e f) -> p e f", e=EC)
                qd = wqkT.ap().rearrange("(e p) f -> p e f", p=128)
                q8s = wqkT8_sb[:].rearrange("p (e f) -> p e f", e=EC)
                q8d = wqkT8.ap().rearrange("(e p) f -> p e f", p=128)
                # DMA order: startup chains first (bf16 wqkT fo 0/3, then xT
                # cols 0:512 per e-chunk so the startup chains pipeline with
                # the DMA stream), then the rest in rough order of first use.
                nc.sync.dma_start(qs[:, :, 0:128], qd[:, :, 0:128])
                nc.sync.dma_start(qs[:, :, 384:512], qd[:, :, 384:512])
                nc.sync.dma_start(xs[:, 0:3, 0:512], xd[:, 0:3, 0:512])
                nc.sync.dma_start(xs[:, 3:6, 0:512], xd[:, 3:6, 0:512])
                nc.sync.dma_start(tri_sb[:], tri.ap())
                nc.sync.dma_start(q8s[:], q8d[:])
                nc.sync.dma_start(x8s[:, :, 0:1024], x8d[:, :, 0:1024])
                nc.sync.dma_start(
                    wvT_sb[:].rearrange("p (e f) -> p e f", e=EC),
                    wvT.ap().rearrange("(e p) f -> p e f", p=128))
                nc.sync.dma_start(tri8_sb[:], tri8.ap())
                nc.sync.dma_start(xs[:, :, 512:S], xd[:, :, 512:S])
                nc.sync.dma_start(x8s[:, :, 1024:S], x8d[:, :, 1024:S])
                nc.sync.dma_start(qs[:, :, 128:384], qd[:, :, 128:384])
                nc.sync.dma_start(qs[:, :, 512:768], qd[:, :, 512:768])
                nc.sync.dma_start(id_sb[:], ident.ap())
                for i in range(3):
                    nc.sync.dma_start(woT_sb[i][:],
                                      woT.ap()[128 * i:128 * (i + 1), :])
                nc.sync.dma_start(bqk_sb[:], bqk.ap())
                nc.sync.dma_start(bv_sb[:], bv.ap())
                nc.sync.dma_start(on_sb[:], ones.ap())
                nc.sync.dma_start(bo_sb[:], bo.ap())

                def emit_qk_chain(fo, on_act=False, cols=(0, 512)):
                    """bf16 qk-proj for f-chunk fo, s-cols [0, 512) only
                    (optionally a sub-range, for the startup k-chain)."""
                    c0, c1 = cols
                    pair = ps_tile([128, 512], "ppro_t", 2)
                    for ecc in range(EC):
                        nc.tensor.matmul(
                            pair[:, c0:c1],
                            wqkT_sb[:, 768 * ecc + 128 * fo:
                                    768 * ecc + 128 * (fo + 1)],
                            xT_sb[:, S * ecc + c0:S * ecc + c1],
                            start=(ecc == 0), stop=(ecc == EC - 1),
                            skip_group_check=True)
                    dst = qkT_sb[fo][:, c0:c1]
                    src = pair[:, c0:c1]
                    if with_bias:
                        nc.vector.tensor_scalar_add(
                            dst, src, bqk_sb[:, fo:fo + 1])
                    elif on_act:
                        nc.scalar.copy(dst, src)
                    else:
                        nc.vector.tensor_copy(dst, src)

                def emit_qk8_chain(fo, win):
                    """fp8 DoubleRow qk-proj for f-chunk fo, one 256-col
                    window. q chunks (fo<3): win 0..5 -> cols 512+256*win.
                    k chunks (fo>=3): win 0..7 -> cols 256*win."""
                    c0 = (512 if fo < 3 else 0) + 256 * win
                    pair = ps_tile([128, 512], "ppro_t", 2)
                    wv = wqkT8_sb[:].rearrange("p (e f) -> p e f", e=EC)
                    xv = xT8_sb[:].rearrange("p (e s) -> p e s", e=EC)
                    for ecp in range(EC // 2):
                        nc.tensor.matmul(
                            pair[:, 0:256],
                            wv[:, 2 * ecp:2 * ecp + 2,
                               128 * fo:128 * (fo + 1)],
                            xv[:, 2 * ecp:2 * ecp + 2, c0:c0 + 256],
                            start=(ecp == 0), stop=(ecp == EC // 2 - 1),
                            skip_group_check=True, perf_mode=DR)
                    dst = qkT8_sb[fo][:, c0:c0 + 256]
                    src = pair[:, 0:256]
                    if with_bias:
                        nc.vector.tensor_scalar_add(
                            dst, src, bqk_sb[:, fo:fo + 1])
                    else:
                        nc.vector.tensor_copy(dst, src)

                def emit_v_chunk(sc):
                    """V-proj for s-chunk sc (bf16), copied out bf16 (sc<4,
                    for the bf16 window) and fp8 (unit-pair layout, always)."""
                    psv = ps_tile([128, FPC], "ppro_t", 2)
                    for ecc in range(EC):
                        nc.tensor.matmul(
                            psv[:],
                            xT_sb[:, S * ecc + 128 * sc:
                                  S * ecc + 128 * (sc + 1)],
                            wvT_sb[:, FPC * ecc:FPC * (ecc + 1)],
                            start=(ecc == 0),
                            stop=(not with_bias and ecc == EC - 1),
                            skip_group_check=True)
                    if with_bias:
                        nc.tensor.matmul(psv[:], on_sb[:, 0:128],
                                         bv_sb[:], start=False, stop=True,
                                         skip_group_check=True)
                    u, t = divmod(sc, 2)
                    v8 = V8_sb[u][:].rearrange("p (t h x) -> p t h x",
                                               t=2, x=65)
                    nc.vector.tensor_copy(
                        v8[:, t, :, 0:64],
                        psv[:].rearrange("p (h x) -> p h x", x=64))
                    nc.gpsimd.memset(v8[:, t, :, 64:65], 1.0)
                    if sc < 4:
                        vv = V_sb[sc][:].rearrange("p (h x) -> p h x", x=65)
                        nc.vector.tensor_copy(
                            vv[:, :, 0:64],
                            psv[:].rearrange("p (h x) -> p h x", x=64))
                        nc.gpsimd.memset(vv[:, :, 64:65], 1.0)

                def emit_scores(hp, qw, u):
                    """Scores S^T[k, q] for a pair of k-chunks, both heads,
                    + exp (+ DVE causal masks on diag blocks). qw=0 runs
                    bf16 (writes Et bf16); qw>=1 runs fp8 (writes Et fp8,
                    exp bias EBIAS). Returns Et [128, 2048]
                    (cols 1024*hd + 512*half + qlocal)."""
                    fp8 = qw >= 1
                    if fp8:
                        qT, kT = qkT8_sb[hp], qkT8_sb[3 + hp]
                        qcol = 512 * qw
                        trit = tri8_sb
                        edt = FP8
                    else:
                        qT, kT = qkT_sb[hp], qkT_sb[3 + hp]
                        qcol = 0       # bf16 tiles hold only cols [0,512)
                        trit = tri_sb
                        edt = BF16
                    pss = {hd: ps_tile([128, 1024], "pss_t", 2)
                           for hd in range(2)}
                    Et = EP.tile([128, 2048], edt, name="E_t")
                    for half in range(2):
                        ki = 2 * u + half
                        j = ki - 4 * qw
                        c = 128 * j if j > 0 else 0
                        # strict row-group alternation (base 0,64,0,64) so
                        # score matmul pairs run on separate PE row groups;
                        # diag tiles shrink the moving window
                        for hd in range(2):
                            base = 64 * hd
                            nc.tensor.matmul(
                                pss[hd][:, 512 * half + c:512 * (half + 1)],
                                kT[base:base + 64, 128 * ki:128 * (ki + 1)],
                                qT[base:base + 64,
                                   qcol + c:qcol + 512],
                                start=True, stop=True,
                                skip_group_check=True)
                    j0 = 2 * u - 4 * qw
                    j1 = j0 + 1
                    c0 = 128 * j0 if j0 > 0 else 0
                    bias = ebias_sb[:, 0:1] if fp8 else 0.0
                    # one exp per head spans both halves when contiguous;
                    # when the half-1 diag shrink leaves an unwritten PSUM
                    # gap, split the exp around it -- EXCEPT in fp8 windows
                    # with a single-block gap (j1 == 1): there one merged exp
                    # over the gap is cheaper than a second instruction. The
                    # gap columns hold stale-but-bounded old scores (the slot
                    # was written by earlier units), their exp is finite, and
                    # the masked-out block is never read by any ctx matmul.
                    for hd in range(2):
                        if fp8 and j1 == 1:
                            nc.scalar.activation(
                                Et[:, 1024 * hd:1024 * (hd + 1)],
                                pss[hd][:, 0:1024], EXP, scale=0.125,
                                bias=bias)
                        elif j1 > 0:
                            nc.scalar.activation(
                                Et[:, 1024 * hd + c0:1024 * hd + 512],
                                pss[hd][:, c0:512], EXP, scale=0.125,
                                bias=bias)
                            c1 = 128 * j1
                            nc.scalar.activation(
                                Et[:, 1024 * hd + 512 + c1:1024 * (hd + 1)],
                                pss[hd][:, 512 + c1:1024], EXP, scale=0.125,
                                bias=bias)
                        else:
                            nc.scalar.activation(
                                Et[:, 1024 * hd + c0:1024 * (hd + 1)],
                                pss[hd][:, c0:1024], EXP, scale=0.125,
                                bias=bias)
                    # causal mask inside the diagonal 128x128 blocks
                    for half in range(2):
                        j = 2 * u + half - 4 * qw
                        if j >= 0:
                            for hd in range(2):
                                off = 1024 * hd + 512 * half + 128 * j
                                nc.vector.tensor_mul(
                                    Et[:, off:off + 128],
                                    Et[:, off:off + 128], trit[:])
                    return Et

                def emit_ctx_qc(hp, qw, qc, Ets, psc):
                    """ctx for one q-chunk, both heads. qw=0: bf16 per-ki
                    matmuls with V_sb. qw>=1: fp8 DoubleRow over k-chunk
                    pairs (units) with V8_sb, plus a plain-fp8 tail when
                    the diagonal cuts a unit in half."""
                    nk = 4 * qw + qc + 1        # k-chunks 0..4qw+qc
                    if qw == 0:
                        for ki in range(nk):
                            u, half = divmod(ki, 2)
                            Et = Ets[u]
                            for hd in range(2):
                                h = 2 * hp + hd
                                nc.tensor.matmul(
                                    psc[hd][:, 65 * qc:65 * qc + 65],
                                    Et[:, 1024 * hd + 512 * half + 128 * qc:
                                        1024 * hd + 512 * half + 128 * qc
                                        + 128],
                                    V_sb[ki][:, 65 * h:65 * h + 65],
                                    start=(ki == 0), stop=(ki == nk - 1),
                                    skip_group_check=True)
                        return
                    nu_full = nk // 2           # full DR unit-pairs
                    tail = nk % 2               # lone half-0 chunk at the end
                    for hd in range(2):
                        h = 2 * hp + hd
                        for u in range(nu_full):
                            ev = Ets[u][:, 1024 * hd:1024 * (hd + 1)] \
                                .rearrange("p (t q) -> p t q", t=2)
                            v8 = V8_sb[u][:].rearrange(
                                "p (t h x) -> p t h x", t=2, x=65)
                            nc.tensor.matmul(
                                psc[hd][:, 65 * qc:65 * qc + 65],
                                ev[:, :, 128 * qc:128 * qc + 128],
                                v8[:, :, h, :],
                                start=(u == 0),
                                stop=(tail == 0 and u == nu_full - 1),
                                skip_group_check=True, perf_mode=DR)
                        if tail:
                            u = nu_full
                            v8 = V8_sb[u][:].rearrange(
                                "p (t h x) -> p t h x", t=2, x=65)
                            nc.tensor.matmul(
                                psc[hd][:, 65 * qc:65 * qc + 65],
                                Ets[u][:, 1024 * hd + 128 * qc:
                                       1024 * hd + 128 * qc + 128],
                                v8[:, 0, h, :],
                                start=(nu_full == 0), stop=True,
                                skip_group_check=True)

                def emit_finish_qc(hp, qw, qc, psc, st, outproj=True):
                    """Per-qc finish for the final step: craw slice + recip +
                    normalize + transpose + ctxT copy + out-proj, so each
                    s-chunk's chain starts the moment its own ctx is done."""
                    if "ctxn" not in st:
                        st["ctxn"] = NP.tile([128, 512], BF16, name="ctxn_t")
                    # psc_t's 2 slots are BOTH live (the psc accumulators)
                    # until the last craw is copied out, so pt must come from
                    # the pss_t ring (scores are done with it by now).
                    pt = ps_tile([128, 512], "pss_t", 2, dtype=BF16)
                    ctxn = st["ctxn"]
                    for hd in range(2):
                        craw = NP.tile([128, 65], F32, name="crawq_t")
                        nc.vector.tensor_copy(
                            craw[:], psc[hd][:, 65 * qc:65 * qc + 65])
                        rinv = NP.tile([128, 1], F32, name="rinvq_t")
                        nc.vector.reciprocal(
                            rinv[:], craw[:, 64:65])
                        nc.vector.tensor_scalar_mul(
                            ctxn[:, 128 * qc + 64 * hd:
                                 128 * qc + 64 * (hd + 1)],
                            craw[:, 0:64], rinv[:])
                    nc.tensor.transpose(
                        pt[:, 128 * qc:128 * (qc + 1)],
                        ctxn[:, 128 * qc:128 * (qc + 1)], id_sb[:])
                    nc.scalar.copy(
                        ctxT_sb[hp][:, 512 * qw + 128 * qc:
                                     512 * qw + 128 * (qc + 1)],
                        pt[:, 128 * qc:128 * (qc + 1)])
                    if outproj:
                        emit_outproj_sc(4 * qw + qc, tail=True)

                def emit_norm_head(hp, qw, psc, st):
                    """Copy raw ctx out of PSUM (freeing psc) and compute
                    reciprocal row-sums."""
                    craws = []
                    for hd in range(2):
                        craw = NP.tile([128, 260], F32, name="craw_t")
                        nc.vector.tensor_copy(craw[:], psc[hd][:])
                        craws.append(craw)
                    pt = ps_tile([128, 512], "psc_t", 2, dtype=BF16)
                    ctxn = NP.tile([128, 512], BF16, name="ctxn_t")
                    rinvs = []
                    for hd in range(2):
                        craw = craws[hd]
                        cv = craw[:].rearrange("p (q x) -> p q x", x=65)
                        rinv = NP.tile([128, 4], F32, name="rinv_t")
                        nc.vector.reciprocal(
                            rinv[:].rearrange("p (q x) -> p q x", x=1),
                            cv[:, :, 64:65])
                        rinvs.append((craw, rinv))
                    st["pt"] = pt
                    st["ctxn"] = ctxn
                    st["rinvs"] = rinvs

                def emit_norm_qc(hp, qw, qc, st):
                    """Normalize + ONE merged 2-head transpose + copy out one
                    128-column ctxT block."""
                    pt, ctxn, rinvs = st["pt"], st["ctxn"], st["rinvs"]
                    for hd in range(2):
                        craw, rinv = rinvs[hd]
                        nc.vector.tensor_scalar_mul(
                            ctxn[:, 128 * qc + 64 * hd:
                                 128 * qc + 64 * (hd + 1)],
                            craw[:, 65 * qc:65 * qc + 64],
                            rinv[:, qc:qc + 1])
                    nc.tensor.transpose(
                        pt[:, 128 * qc:128 * (qc + 1)],
                        ctxn[:, 128 * qc:128 * (qc + 1)], id_sb[:])
                    nc.vector.tensor_copy(
                        ctxT_sb[hp][:, 512 * qw + 128 * qc:
                                     512 * qw + 128 * (qc + 1)],
                        pt[:, 128 * qc:128 * (qc + 1)])

                def emit_outproj_sc(sc, tail=False, ring=None):
                    osb = OP.tile([128, E], F32, name="osb_t")
                    # tail chains run while psc_t's two slots are still live
                    # (the psc accumulators), so they must use pss_t;
                    # explicit ring= alternates banks for back-to-back chains
                    tg, nb = (ring, 2) if ring else \
                        (("pss_t", 2) if tail else ("psc_t", 2))
                    pos = {0: ps_tile([128, 512], tg, nb),
                           512: ps_tile([128, 256], tg, nb)}
                    for c in range(3):
                        for f0, fn in ((0, 512), (512, 256)):
                            nc.tensor.matmul(
                                pos[f0][:, 0:fn],
                                ctxT_sb[c][:, 128 * sc:128 * (sc + 1)],
                                woT_sb[c][:, f0:f0 + fn],
                                start=(c == 0),
                                stop=(not with_bias and c == 2),
                                skip_group_check=True)
                    for f0, fn in ((0, 512), (512, 256)):
                        if with_bias:
                            nc.tensor.matmul(pos[f0][:, 0:fn],
                                             on_sb[:, 0:128],
                                             bo_sb[:, f0:f0 + fn],
                                             start=False, stop=True,
                                             skip_group_check=True)
                        # in the tail ACT is idle (all exps done): put the
                        # copy-outs there and DMA each f-window as it lands
                        if tail:
                            nc.scalar.copy(osb[:, f0:f0 + fn],
                                           pos[f0][:, 0:fn])
                            nc.sync.dma_start(
                                y.ap()[128 * sc:128 * (sc + 1), f0:f0 + fn],
                                osb[:, f0:f0 + fn])
                        else:
                            nc.vector.tensor_copy(osb[:, f0:f0 + fn],
                                                  pos[f0][:, 0:fn])
                    if not tail:
                        nc.sync.dma_start(y.ap()[128 * sc:128 * (sc + 1), :],
                                          osb[:])

                def emit_attention():
                    # software pipeline: ctx runs as per-q-chunk deferred
                    # tasks queued when a step's scores complete; one task
                    # pops per unit so ctx/norm/out-proj spread between
                    # later units while ACT chews on exps.
                    work = []

                    def flush_one():
                        if work:
                            work.pop(0)()
                        if len(work) > 3:
                            work.pop(0)()

                    def make_step(hp, qw, Ets):
                        holder = {}

                        def get_psc():
                            if not holder:
                                holder[0] = {
                                    hd: ps_tile([128, 260], "psc_t", 2)
                                    for hd in range(2)}
                            return holder[0]

                        def ctx_task(qc):
                            return lambda: emit_ctx_qc(
                                hp, qw, qc, Ets, get_psc())
                        return get_psc, ctx_task

                    def finish_step(hp, qw, get_psc):
                        st = {}
                        if hp < 2:
                            def norm_all():
                                emit_norm_head(hp, qw, get_psc(), st)
                                for qc in range(4):
                                    emit_norm_qc(hp, qw, qc, st)
                            work.append(norm_all)
                        else:
                            work.append(lambda: emit_norm_head(
                                hp, qw, get_psc(), st))

                            def norm_op(qc):
                                emit_norm_qc(hp, qw, qc, st)
                                emit_outproj_sc(4 * qw + qc)
                            for qc in range(4):
                                work.append(lambda qc=qc: norm_op(qc))

                    # phase-1 chains interleaved between attention units:
                    # (hp, qw, u) -> thunks emitted right after that unit's
                    # scores+flush. Deadlines:
                    #   bf16 qk chain fo in {hp', 3+hp'} before (hp', 0, 0)
                    #   fp8 k chain (3+hp', win w) before (hp', 1 + w//4,
                    #     u = w % ...) - k-chunks 2w,2w+1 first read at
                    #     (hp', qw >= 1, u = w)
                    #   fp8 q chain (hp', win w) before (hp', 1 + w//2, 0)
                    #   V chunk sc: read by ctx tasks popping ~one step later
                    intra = {}

                    def add(hp, qw, u, fn):
                        intra.setdefault((hp, qw, u), []).append(fn)

                    def addv(hp, qw, u, sc):
                        add(hp, qw, u, lambda: emit_v_chunk(sc))

                    def addqk(hp, qw, u, fo):
                        add(hp, qw, u, lambda: emit_qk_chain(fo))

                    def addq8(hp, qw, u, fo, w):
                        add(hp, qw, u, lambda: emit_qk8_chain(fo, w))

                    # --- hp 0 --- (all 16 V chunks live here; k8(3,*) and
                    # q8(0,*) feed hp0's own fp8 windows)
                    # Step order: plain nested (hp, qw); (2,3) runs
                    # last with the inline early-ctx finish.
                    steps = [(hp, qw) for hp in range(3)
                             for qw in range(QW)]
                    # Placement rule: an intra item at slot (hp, qw, u) is
                    # emitted AFTER scores of unit u+1 (emit-ahead pipeline),
                    # so it may only feed units >= u+2 of its own step.
                    addq8(0, 0, 0, 3, 0)
                    addq8(0, 0, 0, 0, 0)
                    addv(0, 0, 0, 0)
                    addq8(0, 0, 1, 3, 1)
                    addq8(0, 0, 1, 0, 1)
                    addv(0, 0, 1, 1)
                    addv(0, 1, 0, 2)
                    addq8(0, 1, 0, 3, 2)
                    addv(0, 1, 1, 3)
                    addq8(0, 1, 1, 3, 3)
                    addq8(0, 1, 1, 0, 2)
                    addq8(0, 1, 2, 0, 3)
                    addv(0, 1, 3, 4)
                    addq8(0, 1, 3, 3, 4)
                    addv(0, 2, 0, 5)
                    addq8(0, 2, 0, 3, 5)
                    addv(0, 2, 1, 6)
                    addq8(0, 2, 1, 0, 4)
                    addv(0, 2, 2, 7)
                    addq8(0, 2, 3, 0, 5)
                    addv(0, 2, 3, 8)
                    addq8(0, 2, 4, 3, 6)
                    addv(0, 2, 4, 9)
                    addq8(0, 2, 5, 3, 7)
                    addv(0, 3, 0, 10)
                    addv(0, 3, 1, 11)
                    addv(0, 3, 2, 12)
                    addv(0, 3, 3, 13)
                    addv(0, 3, 4, 14)
                    addqk(0, 3, 5, 1)
                    addv(0, 3, 5, 15)
                    addqk(0, 3, 6, 4)
                    addq8(0, 3, 6, 4, 0)
                    addq8(0, 3, 7, 1, 0)
                    addq8(1, 0, 0, 4, 1)
                    addq8(1, 0, 0, 1, 1)
                    addq8(1, 0, 1, 4, 2)
                    addq8(1, 1, 0, 4, 3)
                    addq8(1, 1, 1, 1, 2)
                    addq8(1, 1, 2, 1, 3)
                    addq8(1, 1, 3, 4, 4)
                    addq8(1, 2, 0, 4, 5)
                    addq8(1, 2, 1, 1, 4)
                    addq8(1, 2, 3, 1, 5)
                    addq8(1, 2, 4, 4, 6)
                    addq8(1, 2, 5, 4, 7)
                    addqk(1, 3, 1, 2)
                    addqk(1, 3, 2, 5)
                    addq8(1, 3, 3, 5, 0)
                    addq8(1, 3, 4, 5, 1)
                    addq8(1, 3, 5, 2, 0)
                    addq8(1, 3, 6, 2, 1)
                    addq8(1, 3, 7, 5, 2)
                    addq8(2, 0, 0, 5, 3)
                    addq8(2, 0, 1, 2, 2)
                    addq8(2, 1, 0, 5, 4)
                    addq8(2, 1, 1, 2, 3)
                    addq8(2, 1, 2, 5, 5)
                    addq8(2, 2, 0, 5, 6)
                    addq8(2, 2, 1, 2, 4)
                    addq8(2, 2, 3, 2, 5)
                    addq8(2, 2, 4, 5, 7)

                    for si, (hp, qw) in enumerate(steps):
                        nu = 2 * qw + 2
                        last = (si == len(steps) - 1)
                        Ets = []
                        get_psc, ctx_task = make_step(hp, qw, Ets)
                        if last:
                            # run all but the final two units normally (with
                            # harder draining), then emit the last two units'
                            # scores up front so ACT stays fed while the PE
                            # drains the backlog and runs the tail chains
                            Ets.append(emit_scores(hp, qw, 0))
                            for u in range(nu - 2):
                                if u + 1 < nu - 2:
                                    Ets.append(emit_scores(hp, qw, u + 1))
                                for fn in intra.get((hp, qw, u), ()):
                                    fn()
                                flush_one()
                                flush_one()
                            Ets.append(emit_scores(hp, qw, nu - 2))
                            Ets.append(emit_scores(hp, qw, nu - 1))
                            while work:
                                work.pop(0)()
                            st = {}
                            for qc in range(4):
                                ctx_task(qc)()
                            for qc in range(4):
                                emit_finish_qc(hp, qw, qc, get_psc(), st,
                                               outproj=False)
                            for qc in range(4):
                                emit_outproj_sc(4 * qw + qc, tail=True)
                            continue
                        # emit-ahead software pipeline                        # emit-ahead software pipeline: unit u+1's scores go
                        # out BEFORE unit u's intra chains / deferred pops, so
                        # the next exps are never queued behind filler work on
                        # the in-order PE stream.
                        Ets.append(emit_scores(hp, qw, 0))
                        for u in range(nu):
                            if u + 1 < nu:
                                Ets.append(emit_scores(hp, qw, u + 1))
                            for fn in intra.get((hp, qw, u), ()):
                                fn()
                            flush_one()
                        for qc in range(4):
                            work.append(ctx_task(qc))
                        finish_step(hp, qw, get_psc)
                    while work:
                        flush_one()

                # PE p-state warmup: dummy matmuls on a memset tile keep the
                # PE busy through the startup DMAs so the real startup chains
                # run at a higher clock (the cost model ramps 0.65 -> 1.2 ->
                # 2.4 GHz with continuous execution).
                warm_sb = PI.tile([128, 128], BF16, name="warm_sb")
                nc.gpsimd.memset(warm_sb[:], 0.5)
                wps = ps_tile([128, 512], "ppro_t", 2)
                for _ in range(24):
                    nc.tensor.matmul(wps[:, 0:128], warm_sb[:], warm_sb[:],
                                     start=True, stop=True,
                                     skip_group_check=True)

                # start-up: only what the first scores need
                emit_qk_chain(0)
                emit_qk_chain(3, on_act=True)
                emit_attention()

    return _patch_multiwait(nc)


_NC = {}


def _get_nc(with_bias=True):
    if with_bias not in _NC:
        _NC[with_bias] = build_nc(with_bias=with_bias)
    return _NC[with_bias]


def _prep_core_inputs(x, in_proj_w, in_proj_b, out_w, out_b):
    """Build the 8 per-core input dicts (host-side shard + transpose)."""
    import ml_dtypes
    bf16 = ml_dtypes.bfloat16
    fp8 = ml_dtypes.float8_e4m3
    # 0/1 keep-mask for S^T[k, q] diagonal blocks: keep where k <= q
    tri_np = (np.arange(128)[:, None] <= np.arange(128)[None, :])
    tri_bf = tri_np.astype(bf16)
    tri_f8 = tri_np.astype(fp8)
    id_bf = np.eye(128, dtype=np.float32).astype(bf16)
    ones_np = np.ones((1, 128), np.float32).astype(bf16)

    xT_by_b = [np.ascontiguousarray(np.asarray(x[b]).T) for b in range(B)]
    xT_bf = [a.astype(bf16) for a in xT_by_b]
    xT_f8 = [a.astype(fp8) for a in xT_by_b]

    in_maps = []
    for c in range(8):
        b = c // 2
        g = c % 2
        f0 = FPC * g
        Wq = np.asarray(in_proj_w[f0:f0 + FPC])
        Wk = np.asarray(in_proj_w[E + f0:E + f0 + FPC])
        Wv = np.asarray(in_proj_w[2 * E + f0:2 * E + f0 + FPC])
        bq = np.asarray(in_proj_b[f0:f0 + FPC])
        bk = np.asarray(in_proj_b[E + f0:E + f0 + FPC])
        bvv = np.asarray(in_proj_b[2 * E + f0:2 * E + f0 + FPC])
        Wo = np.asarray(out_w[:, f0:f0 + FPC])
        bqk_np = np.concatenate([bq, bk]).astype(np.float32).reshape(6, 128).T
        wqkT_np = np.ascontiguousarray(
            np.concatenate([Wq, Wk], axis=0).T).astype(np.float32)
        in_maps.append({
            "xT": xT_bf[b],
            "xT8": xT_f8[b],
            "wqkT": wqkT_np.astype(bf16),
            "wqkT8": wqkT_np.astype(fp8),
            "wvT": np.ascontiguousarray(Wv.T).astype(bf16),
            "woT": np.ascontiguousarray(Wo.T).astype(bf16),
            "bqk": np.ascontiguousarray(bqk_np),
            "bv": bvv.reshape(1, FPC).astype(bf16),
            # out bias only on even cores so the host-side pair-sum is exact
            "bo": np.asarray(out_b).reshape(1, E).astype(bf16) if g == 0
                  else np.zeros((1, E), bf16),
            "tri": tri_bf,
            "tri8": tri_f8,
            "ident": id_bf,
            "ones": ones_np,
        })
    return in_maps


def kernel(x, in_proj_w, in_proj_b, out_w, out_b):
    zero_bias = (not np.any(np.asarray(in_proj_b))) and \
                (not np.any(np.asarray(out_b)))
    nc = _get_nc(with_bias=not zero_bias)
    in_maps = _prep_core_inputs(x, in_proj_w, in_proj_b, out_w, out_b)
    res = run_bass_kernel_spmd(nc, in_maps, core_ids=list(range(8)))
    out = np.empty((B, S, E), np.float32)
    for b in range(B):
        out[b] = res.results[2 * b]["y"] + res.results[2 * b + 1]["y"]
    return out


# revision 39
# speedup vs baseline: 1.0232x; 1.0014x over previous
"""Multi-head attention (B=4, S=2048, E=768, H=12, D=64, causal) on 8 trn2
NeuronCores.

Sharding: core c -> batch b = c//2, head-half g = c%2 (6 heads each).
Each core computes its 6 heads' attention plus the partial output
projection; the host sums the two half-head partials per batch.

Hybrid precision (validated against the fp32 reference in numpy):
  - Rows q < 512 (q-window 0) have concentrated softmax (few keys), so
    their whole attention path stays bf16: bf16 qk-projection (only the
    first 512 q/k columns are needed causally), bf16 scores, bf16 exp'd
    weights, bf16 ctx with bf16 V.
  - Rows q >= 512 average fp8 quantization noise over many keys: fp8e4
    q/k (from an fp8 DoubleRow qk-projection over e-pairs), plain-fp8
    score matmuls, exp'd weights written fp8e4 by ACT (bias -2.5 keeps
    them in fp8 range; softmax normalization cancels the shift), and the
    ctx matmul runs fp8 DoubleRow over k-chunk pairs (half the
    instructions, quarter the PE cycles of bf16).
  Combined max-rel-err ~8e-3 vs the 2e-2 gate.

On-device strategy (per core) otherwise matches the bf16 design:
  - Host pre-transposes x / weight slices so every contraction dim lands
    on SBUF partitions; x ships in bf16 and fp8.
  - Scores are computed TRANSPOSED (S^T[k, q] = K^T Q); diagonal tiles
    shrink their moving window; in-diagonal-block causal masking is a
    DVE multiply by a 0/1 lower-triangle.
  - V_aug packs a ones column per head so the PE emits softmax row-sums
    for free; V is stored bf16 (k-chunks 0-3, for the bf16 window) and
    fp8 in per-unit pair layout [128, 2, 65*HPC] for DoubleRow.
  - ctx lands [q, d+1] with q on partitions; normalization is a DVE
    reciprocal + per-qc scalar multiply; ONE merged [128,128] PE
    transpose per (hp, qw, qc) brings both heads' ctx^T back.
  - The attention phase is ACT(exp)-throughput-bound; projections and
    ctx/norm/out-proj run as deferred tasks interleaved between units.
"""
import sys, json, os

for _p in ("/opt/trn_rl_repo",):
    if _p not in sys.path and os.path.isdir(_p):
        sys.path.insert(0, _p)

import numpy as np
import concourse.bass as bass
import concourse.mybir as mybir
import concourse.tile as tile
from concourse.bass_utils import run_bass_kernel_spmd

B, S, E, H, D = 4, 2048, 768, 12, 64
HPC = H // 2          # heads per core = 6
FPC = HPC * D         # features per core per q/k/v = 384
EC = E // 128         # 6 contraction chunks for projections
SC = S // 128         # 16 s-chunks
QW = S // 512         # 4 q-windows
KC = S // 128         # 16 k-chunks
F32 = mybir.dt.float32
BF16 = mybir.dt.bfloat16
FP8 = mybir.dt.float8e4
EXP = mybir.ActivationFunctionType.Exp
DR = mybir.MatmulPerfMode.DoubleRow
EBIAS = -2.5          # exp bias for fp8 windows (softmax-invariant)


def _patch_multiwait(nc, max_waits=1):
    """This container's walrus rejects instructions with more than one sync
    wait. Split excess waits onto same-engine NOPs emitted immediately
    before the instruction (same-engine streams are order-preserving)."""
    raw = nc.to_json_bytes()
    m = json.loads(raw)
    for f in m["functions"]:
        for b in f["blocks"]:
            out = []
            for inst in b["instructions"]:
                si = inst.get("sync_info") or {}
                ws = si.get("on_wait") or []
                if len(ws) > max_waits:
                    eng = inst["engine"]
                    for i, w in enumerate(ws[:-max_waits]):
                        out.append({
                            "debug": inst.get("debug", 0), "engine": eng,
                            "ins": [], "name": inst["name"] + f"-mw{i}",
                            "opcode": "NoOp", "outs": [],
                            "sync_info": {"on_update": [], "on_wait": [w]},
                        })
                    si["on_wait"] = ws[-max_waits:]
                out.append(inst)
            b["instructions"] = out
    patched = json.dumps(m).encode()
    nc.to_json_bytes = lambda: patched
    return nc


def build_nc(with_bias=True):
    nc = bass.Bass()
    xT = nc.dram_tensor("xT", [E, S], BF16, kind="ExternalInput")
    xT8 = nc.dram_tensor("xT8", [E, S], FP8, kind="ExternalInput")
    wqkT = nc.dram_tensor("wqkT", [E, 2 * FPC], BF16, kind="ExternalInput")
    wqkT8 = nc.dram_tensor("wqkT8", [E, 2 * FPC], FP8, kind="ExternalInput")
    wvT = nc.dram_tensor("wvT", [E, FPC], BF16, kind="ExternalInput")
    woT = nc.dram_tensor("woT", [FPC, E], BF16, kind="ExternalInput")
    bqk = nc.dram_tensor("bqk", [128, 2 * FPC // 128], F32, kind="ExternalInput")
    bv = nc.dram_tensor("bv", [1, FPC], BF16, kind="ExternalInput")
    bo = nc.dram_tensor("bo", [1, E], BF16, kind="ExternalInput")
    tri = nc.dram_tensor("tri", [128, 128], BF16, kind="ExternalInput")
    tri8 = nc.dram_tensor("tri8", [128, 128], FP8, kind="ExternalInput")
    ident = nc.dram_tensor("ident", [128, 128], BF16, kind="ExternalInput")
    ones = nc.dram_tensor("ones", [1, 128], BF16, kind="ExternalInput")
    y = nc.dram_tensor("y", [S, E], F32, kind="ExternalOutput")

    with tile.TileContext(nc) as tc, \
         nc.allow_low_precision(reason="hybrid bf16/fp8 pipeline by design"):
        with tc.tile_pool(name="persist", bufs=1) as P, \
             tc.tile_pool(name="ps", bufs=1, space="PSUM") as PS:
            # --- persistent tiles (bottom-of-stack, live whole kernel)
            # bf16 q/k: only columns [0, 512) are ever read (q-window 0)
            qkT_sb = [P.tile([128, 512], BF16, name=f"qkT{i}") for i in range(6)]
            # fp8 q/k: q chunks hold cols [512, 2048), k chunks [0, 2048)
            qkT8_sb = [P.tile([128, S], FP8, name=f"qkT8_{i}") for i in range(6)]
            # bf16 V for the bf16 window's ctx (k-chunks 0..3 only)
            V_sb = [P.tile([128, 65 * HPC], BF16, name=f"V{i}") for i in range(4)]
            # fp8 V in unit-pair layout: per partition [t(2), h(6), 65]
            V8_sb = [P.tile([128, 2 * 65 * HPC], FP8, name=f"V8_{i}")
                     for i in range(KC // 2)]
            ctxT_sb = [P.tile([128, S], BF16, name=f"ctxT{i}") for i in range(3)]
            woT_sb = [P.tile([128, E], BF16, name=f"woT{i}") for i in range(3)]
            bqk_sb = P.tile([128, 6], F32, name="bqk_sb")
            bv_sb = P.tile([1, FPC], BF16, name="bv_sb")
            bo_sb = P.tile([1, E], BF16, name="bo_sb")
            tri_sb = P.tile([128, 128], BF16, name="tri_sb")
            tri8_sb = P.tile([128, 128], FP8, name="tri8_sb")
            id_sb = P.tile([128, 128], BF16, name="id_sb")
            on_sb = P.tile([1, 128], BF16, name="on_sb")
            ebias_sb = P.tile([128, 1], F32, name="ebias_sb")
            nc.gpsimd.memset(ebias_sb[:], EBIAS)

            def ps_tile(shape, tag, bufs, dtype=F32):
                return PS.tile(shape, dtype, name=tag, tag=tag, bufs=bufs)

            # ============ phase 1 (projections) + attention, interleaved ====
            with tc.tile_pool(name="inp", bufs=1) as PI, \
                 tc.tile_pool(name="esb", bufs=14) as EP, \
                 tc.tile_pool(name="nrm", bufs=12) as NP, \
                 tc.tile_pool(name="osb", bufs=3) as OP:
                # consolidated input tiles; one DMA dispatch per tensor region
                xT_sb = PI.tile([128, EC * S], BF16, name="xT_all")
                xT8_sb = PI.tile([128, EC * S], FP8, name="xT8_all")
                wqkT_sb = PI.tile([128, EC * 2 * FPC], BF16, name="wqkT_all")
                wqkT8_sb = PI.tile([128, EC * 2 * FPC], FP8, name="wqkT8_all")
                wvT_sb = PI.tile([128, EC * FPC], BF16, name="wvT_all")
                xs = xT_sb[:].rearrange("p (e s) -> p e s", e=EC)
                xd = xT.ap().rearrange("(e p) s -> p e s", p=128)
                x8s = xT8_sb[:].rearrange("p (e s) -> p e s", e=EC)
                x8d = xT8.ap().rearrange("(e p) s -> p e s", p=128)
                qs = wqkT_sb[:].rearrange("p (e f) -> p e f", e=EC)
                qd = wqkT.ap().rearrange("(e p) f -> p e f", p=128)
                q8s = wqkT8_sb[:].rearrange("p (e f) -> p e f", e=EC)
                q8d = wqkT8.ap().rearrange("(e p) f -> p e f", p=128)
                # DMA order: startup chains first (bf16 wqkT fo 0/3, then xT
                # cols 0:512 per e-chunk so the startup chains pipeline with
                # the DMA stream), then the rest in rough order of first use.
                nc.sync.dma_start(xs[:, 0:3, 0:512], xd[:, 0:3, 0:512])
                nc.sync.dma_start(qs[:, :, 0:128], qd[:, :, 0:128])
                nc.sync.dma_start(qs[:, :, 384:512], qd[:, :, 384:512])
                nc.sync.dma_start(xs[:, 3:6, 0:512], xd[:, 3:6, 0:512])
                nc.sync.dma_start(tri_sb[:], tri.ap())
                nc.sync.dma_start(q8s[:], q8d[:])
                nc.sync.dma_start(x8s[:, :, 0:1024], x8d[:, :, 0:1024])
                nc.sync.dma_start(
                    wvT_sb[:].rearrange("p (e f) -> p e f", e=EC),
                    wvT.ap().rearrange("(e p) f -> p e f", p=128))
                nc.sync.dma_start(tri8_sb[:], tri8.ap())
                nc.sync.dma_start(xs[:, :, 512:S], xd[:, :, 512:S])
                nc.sync.dma_start(x8s[:, :, 1024:S], x8d[:, :, 1024:S])
                nc.sync.dma_start(qs[:, :, 128:384], qd[:, :, 128:384])
                nc.sync.dma_start(qs[:, :, 512:768], qd[:, :, 512:768])
                nc.sync.dma_start(id_sb[:], ident.ap())
                for i in range(3):
                    nc.sync.dma_start(woT_sb[i][:],
                                      woT.ap()[128 * i:128 * (i + 1), :])
                nc.sync.dma_start(bqk_sb[:], bqk.ap())
                nc.sync.dma_start(bv_sb[:], bv.ap())
                nc.sync.dma_start(on_sb[:], ones.ap())
                nc.sync.dma_start(bo_sb[:], bo.ap())

                def emit_qk_chain(fo, on_act=False, cols=(0, 512)):
                    """bf16 qk-proj for f-chunk fo, s-cols [0, 512) only
                    (optionally a sub-range, for the startup k-chain)."""
                    c0, c1 = cols
                    pair = ps_tile([128, 512], "ppro_t", 2)
                    for ecc in range(EC):
                        nc.tensor.matmul(
                            pair[:, c0:c1],
                            wqkT_sb[:, 768 * ecc + 128 * fo:
                                    768 * ecc + 128 * (fo + 1)],
                            xT_sb[:, S * ecc + c0:S * ecc + c1],
                            start=(ecc == 0), stop=(ecc == EC - 1),
                            skip_group_check=True)
                    dst = qkT_sb[fo][:, c0:c1]
                    src = pair[:, c0:c1]
                    if with_bias:
                        nc.vector.tensor_scalar_add(
                            dst, src, bqk_sb[:, fo:fo + 1])
                    elif on_act:
                        nc.scalar.copy(dst, src)
                    else:
                        nc.vector.tensor_copy(dst, src)

                def emit_qk8_chain(fo, win):
                    """fp8 DoubleRow qk-proj for f-chunk fo, one 256-col
                    window. q chunks (fo<3): win 0..5 -> cols 512+256*win.
                    k chunks (fo>=3): win 0..7 -> cols 256*win."""
                    c0 = (512 if fo < 3 else 0) + 256 * win
                    pair = ps_tile([128, 512], "ppro_t", 2)
                    wv = wqkT8_sb[:].rearrange("p (e f) -> p e f", e=EC)
                    xv = xT8_sb[:].rearrange("p (e s) -> p e s", e=EC)
                    for ecp in range(EC // 2):
                        nc.tensor.matmul(
                            pair[:, 0:256],
                            wv[:, 2 * ecp:2 * ecp + 2,
                               128 * fo:128 * (fo + 1)],
                            xv[:, 2 * ecp:2 * ecp + 2, c0:c0 + 256],
                            start=(ecp == 0), stop=(ecp == EC // 2 - 1),
                            skip_group_check=True, perf_mode=DR)
                    dst = qkT8_sb[fo][:, c0:c0 + 256]
                    src = pair[:, 0:256]
                    if with_bias:
                        nc.vector.tensor_scalar_add(
                            dst, src, bqk_sb[:, fo:fo + 1])
                    else:
                        nc.vector.tensor_copy(dst, src)

                def emit_v_chunk(sc):
                    """V-proj for s-chunk sc (bf16), copied out bf16 (sc<4,
                    for the bf16 window) and fp8 (unit-pair layout, always)."""
                    psv = ps_tile([128, FPC], "ppro_t", 2)
                    for ecc in range(EC):
                        nc.tensor.matmul(
                            psv[:],
                            xT_sb[:, S * ecc + 128 * sc:
                                  S * ecc + 128 * (sc + 1)],
                            wvT_sb[:, FPC * ecc:FPC * (ecc + 1)],
                            start=(ecc == 0),
                            stop=(not with_bias and ecc == EC - 1),
                            skip_group_check=True)
                    if with_bias:
                        nc.tensor.matmul(psv[:], on_sb[:, 0:128],
                                         bv_sb[:], start=False, stop=True,
                                         skip_group_check=True)
                    u, t = divmod(sc, 2)
                    v8 = V8_sb[u][:].rearrange("p (t h x) -> p t h x",
                                               t=2, x=65)
                    nc.vector.tensor_copy(
                        v8[:, t, :, 0:64],
                        psv[:].rearrange("p (h x) -> p h x", x=64))
                    nc.gpsimd.memset(v8[:, t, :, 64:65], 1.0)
                    if sc < 4:
                        vv = V_sb[sc][:].rearrange("p (h x) -> p h x", x=65)
                        nc.vector.tensor_copy(
                            vv[:, :, 0:64],
                            psv[:].rearrange("p (h x) -> p h x", x=64))
                        nc.gpsimd.memset(vv[:, :, 64:65], 1.0)

                def emit_scores(hp, qw, u):
                    """Scores S^T[k, q] for a pair of k-chunks, both heads,
                    + exp (+ DVE causal masks on diag blocks). qw=0 runs
                    bf16 (writes Et bf16); qw>=1 runs fp8 (writes Et fp8,
                    exp bias EBIAS). Returns Et [128, 2048]
                    (cols 1024*hd + 512*half + qlocal)."""
                    fp8 = qw >= 1
                    if fp8:
                        qT, kT = qkT8_sb[hp], qkT8_sb[3 + hp]
                        qcol = 512 * qw
                        trit = tri8_sb
                        edt = FP8
                    else:
                        qT, kT = qkT_sb[hp], qkT_sb[3 + hp]
                        qcol = 0       # bf16 tiles hold only cols [0,512)
                        trit = tri_sb
                        edt = BF16
                    pss = {hd: ps_tile([128, 1024], "pss_t", 2)
                           for hd in range(2)}
                    Et = EP.tile([128, 2048], edt, name="E_t")
                    for half in range(2):
                        ki = 2 * u + half
                        j = ki - 4 * qw
                        c = 128 * j if j > 0 else 0
                        # strict row-group alternation (base 0,64,0,64) so
                        # score matmul pairs run on separate PE row groups;
                        # diag tiles shrink the moving window
                        for hd in range(2):
                            base = 64 * hd
                            nc.tensor.matmul(
                                pss[hd][:, 512 * half + c:512 * (half + 1)],
                                kT[base:base + 64, 128 * ki:128 * (ki + 1)],
                                qT[base:base + 64,
                                   qcol + c:qcol + 512],
                                start=True, stop=True,
                                skip_group_check=True)
                    j0 = 2 * u - 4 * qw
                    j1 = j0 + 1
                    c0 = 128 * j0 if j0 > 0 else 0
                    bias = ebias_sb[:, 0:1] if fp8 else 0.0
                    # one exp per head spans both halves when contiguous;
                    # when the half-1 diag shrink leaves an unwritten PSUM
                    # gap, split the exp around it -- EXCEPT in fp8 windows
                    # with a single-block gap (j1 == 1): there one merged exp
                    # over the gap is cheaper than a second instruction. The
                    # gap columns hold stale-but-bounded old scores (the slot
                    # was written by earlier units), their exp is finite, and
                    # the masked-out block is never read by any ctx matmul.
                    for hd in range(2):
                        if fp8 and j1 == 1:
                            nc.scalar.activation(
                                Et[:, 1024 * hd:1024 * (hd + 1)],
                                pss[hd][:, 0:1024], EXP, scale=0.125,
                                bias=bias)
                        elif j1 > 0:
                            nc.scalar.activation(
                                Et[:, 1024 * hd + c0:1024 * hd + 512],
                                pss[hd][:, c0:512], EXP, scale=0.125,
                                bias=bias)
                            c1 = 128 * j1
                            nc.scalar.activation(
                                Et[:, 1024 * hd + 512 + c1:1024 * (hd + 1)],
                                pss[hd][:, 512 + c1:1024], EXP, scale=0.125,
                                bias=bias)
                        else:
                            nc.scalar.activation(
                                Et[:, 1024 * hd + c0:1024 * (hd + 1)],
                                pss[hd][:, c0:1024], EXP, scale=0.125,
                                bias=bias)
                    # causal mask inside the diagonal 128x128 blocks
                    for half in range(2):
                        j = 2 * u + half - 4 * qw
                        if j >= 0:
                            for hd in range(2):
                                off = 1024 * hd + 512 * half + 128 * j
                                nc.vector.tensor_mul(
                                    Et[:, off:off + 128],
                                    Et[:, off:off + 128], trit[:])
                    return Et

                def emit_ctx_qc(hp, qw, qc, Ets, psc):
                    """ctx for one q-chunk, both heads. qw=0: bf16 per-ki
                    matmuls with V_sb. qw>=1: fp8 DoubleRow over k-chunk
                    pairs (units) with V8_sb, plus a plain-fp8 tail when
                    the diagonal cuts a unit in half."""
                    nk = 4 * qw + qc + 1        # k-chunks 0..4qw+qc
                    if qw == 0:
                        for ki in range(nk):
                            u, half = divmod(ki, 2)
                            Et = Ets[u]
                            for hd in range(2):
                                h = 2 * hp + hd
                                nc.tensor.matmul(
                                    psc[hd][:, 65 * qc:65 * qc + 65],
                                    Et[:, 1024 * hd + 512 * half + 128 * qc:
                                        1024 * hd + 512 * half + 128 * qc
                                        + 128],
                                    V_sb[ki][:, 65 * h:65 * h + 65],
                                    start=(ki == 0), stop=(ki == nk - 1),
                                    skip_group_check=True)
                        return
                    nu_full = nk // 2           # full DR unit-pairs
                    tail = nk % 2               # lone half-0 chunk at the end
                    for hd in range(2):
                        h = 2 * hp + hd
                        for u in range(nu_full):
                            ev = Ets[u][:, 1024 * hd:1024 * (hd + 1)] \
                                .rearrange("p (t q) -> p t q", t=2)
                            v8 = V8_sb[u][:].rearrange(
                                "p (t h x) -> p t h x", t=2, x=65)
                            nc.tensor.matmul(
                                psc[hd][:, 65 * qc:65 * qc + 65],
                                ev[:, :, 128 * qc:128 * qc + 128],
                                v8[:, :, h, :],
                                start=(u == 0),
                                stop=(tail == 0 and u == nu_full - 1),
                                skip_group_check=True, perf_mode=DR)
                        if tail:
                            u = nu_full
                            v8 = V8_sb[u][:].rearrange(
                                "p (t h x) -> p t h x", t=2, x=65)
                            nc.tensor.matmul(
                                psc[hd][:, 65 * qc:65 * qc + 65],
                                Ets[u][:, 1024 * hd + 128 * qc:
                                       1024 * hd + 128 * qc + 128],
                                v8[:, 0, h, :],
                                start=(nu_full == 0), stop=True,
                                skip_group_check=True)

                def emit_finish_qc(hp, qw, qc, psc, st, outproj=True):
                    """Per-qc finish for the final step: craw slice + recip +
                    normalize + transpose + ctxT copy + out-proj, so each
                    s-chunk's chain starts the moment its own ctx is done."""
                    if "ctxn" not in st:
                        st["ctxn"] = NP.tile([128, 512], BF16, name="ctxn_t")
                    # psc_t's 2 slots are BOTH live (the psc accumulators)
                    # until the last craw is copied out, so pt must come from
                    # the pss_t ring (scores are done with it by now).
                    pt = ps_tile([128, 512], "pss_t", 2, dtype=BF16)
                    ctxn = st["ctxn"]
                    for hd in range(2):
                        craw = NP.tile([128, 65], F32, name="crawq_t")
                        nc.vector.tensor_copy(
                            craw[:], psc[hd][:, 65 * qc:65 * qc + 65])
                        rinv = NP.tile([128, 1], F32, name="rinvq_t")
                        nc.vector.reciprocal(
                            rinv[:], craw[:, 64:65])
                        nc.vector.tensor_scalar_mul(
                            ctxn[:, 128 * qc + 64 * hd:
                                 128 * qc + 64 * (hd + 1)],
                            craw[:, 0:64], rinv[:])
                    nc.tensor.transpose(
                        pt[:, 128 * qc:128 * (qc + 1)],
                        ctxn[:, 128 * qc:128 * (qc + 1)], id_sb[:])
                    nc.scalar.copy(
                        ctxT_sb[hp][:, 512 * qw + 128 * qc:
                                     512 * qw + 128 * (qc + 1)],
                        pt[:, 128 * qc:128 * (qc + 1)])
                    if outproj:
                        emit_outproj_sc(4 * qw + qc, tail=True)

                def emit_norm_head(hp, qw, psc, st):
                    """Copy raw ctx out of PSUM (freeing psc) and compute
                    reciprocal row-sums."""
                    craws = []
                    for hd in range(2):
                        craw = NP.tile([128, 260], F32, name="craw_t")
                        nc.vector.tensor_copy(craw[:], psc[hd][:])
                        craws.append(craw)
                    pt = ps_tile([128, 512], "psc_t", 2, dtype=BF16)
                    ctxn = NP.tile([128, 512], BF16, name="ctxn_t")
                    rinvs = []
                    for hd in range(2):
                        craw = craws[hd]
                        cv = craw[:].rearrange("p (q x) -> p q x", x=65)
                        rinv = NP.tile([128, 4], F32, name="rinv_t")
                        nc.vector.reciprocal(
                            rinv[:].rearrange("p (q x) -> p q x", x=1),
                            cv[:, :, 64:65])
                        rinvs.append((craw, rinv))
                    st["pt"] = pt
                    st["ctxn"] = ctxn
                    st["rinvs"] = rinvs

                def emit_norm_qc(hp, qw, qc, st):
                    """Normalize + ONE merged 2-head transpose + copy out one
                    128-column ctxT block."""
                    pt, ctxn, rinvs = st["pt"], st["ctxn"], st["rinvs"]
                    for hd in range(2):
                        craw, rinv = rinvs[hd]
                        nc.vector.tensor_scalar_mul(
                            ctxn[:, 128 * qc + 64 * hd:
                                 128 * qc + 64 * (hd + 1)],
                            craw[:, 65 * qc:65 * qc + 64],
                            rinv[:, qc:qc + 1])
                    nc.tensor.transpose(
                        pt[:, 128 * qc:128 * (qc + 1)],
                        ctxn[:, 128 * qc:128 * (qc + 1)], id_sb[:])
                    nc.vector.tensor_copy(
                        ctxT_sb[hp][:, 512 * qw + 128 * qc:
                                     512 * qw + 128 * (qc + 1)],
                        pt[:, 128 * qc:128 * (qc + 1)])

                def emit_outproj_sc(sc, tail=False, ring=None):
                    osb = OP.tile([128, E], F32, name="osb_t")
                    # tail chains run while psc_t's two slots are still live
                    # (the psc accumulators), so they must use pss_t;
                    # explicit ring= alternates banks for back-to-back chains
                    tg, nb = (ring, 2) if ring else \
                        (("pss_t", 2) if tail else ("psc_t", 2))
                    pos = {0: ps_tile([128, 512], tg, nb),
                           512: ps_tile([128, 256], tg, nb)}
                    for c in range(3):
                        for f0, fn in ((0, 512), (512, 256)):
                            nc.tensor.matmul(
                                pos[f0][:, 0:fn],
                                ctxT_sb[c][:, 128 * sc:128 * (sc + 1)],
                                woT_sb[c][:, f0:f0 + fn],
                                start=(c == 0),
                                stop=(not with_bias and c == 2),
                                skip_group_check=True)
                    for f0, fn in ((0, 512), (512, 256)):
                        if with_bias:
                            nc.tensor.matmul(pos[f0][:, 0:fn],
                                             on_sb[:, 0:128],
                                             bo_sb[:, f0:f0 + fn],
                                             start=False, stop=True,
                                             skip_group_check=True)
                        # in the tail ACT is idle (all exps done): put the
                        # copy-outs there and DMA each f-window as it lands
                        if tail:
                            nc.scalar.copy(osb[:, f0:f0 + fn],
                                           pos[f0][:, 0:fn])
                            nc.sync.dma_start(
                                y.ap()[128 * sc:128 * (sc + 1), f0:f0 + fn],
                                osb[:, f0:f0 + fn])
                        else:
                            nc.vector.tensor_copy(osb[:, f0:f0 + fn],
                                                  pos[f0][:, 0:fn])
                    if not tail:
                        nc.sync.dma_start(y.ap()[128 * sc:128 * (sc + 1), :],
                                          osb[:])

                def emit_attention():
                    # software pipeline: ctx runs as per-q-chunk deferred
                    # tasks queued when a step's scores complete; one task
                    # pops per unit so ctx/norm/out-proj spread between
                    # later units while ACT chews on exps.
                    work = []

                    def flush_one():
                        if work:
                            work.pop(0)()
                        if len(work) > 3:
                            work.pop(0)()

                    def make_step(hp, qw, Ets):
                        holder = {}

                        def get_psc():
                            if not holder:
                                holder[0] = {
                                    hd: ps_tile([128, 260], "psc_t", 2)
                                    for hd in range(2)}
                            return holder[0]

                        def ctx_task(qc):
                            return lambda: emit_ctx_qc(
                                hp, qw, qc, Ets, get_psc())
                        return get_psc, ctx_task

                    def finish_step(hp, qw, get_psc):
                        st = {}
                        if hp < 2:
                            def norm_all():
                                emit_norm_head(hp, qw, get_psc(), st)
                                for qc in range(4):
                                    emit_norm_qc(hp, qw, qc, st)
                            work.append(norm_all)
                        else:
                            work.append(lambda: emit_norm_head(
                                hp, qw, get_psc(), st))

                            def norm_op(qc):
                                emit_norm_qc(hp, qw, qc, st)
                                emit_outproj_sc(4 * qw + qc)
                            for qc in range(4):
                                work.append(lambda qc=qc: norm_op(qc))

                    # phase-1 chains interleaved between attention units:
                    # (hp, qw, u) -> thunks emitted right after that unit's
                    # scores+flush. Deadlines:
                    #   bf16 qk chain fo in {hp', 3+hp'} before (hp', 0, 0)
                    #   fp8 k chain (3+hp', win w) before (hp', 1 + w//4,
                    #     u = w % ...) - k-chunks 2w,2w+1 first read at
                    #     (hp', qw >= 1, u = w)
                    #   fp8 q chain (hp', win w) before (hp', 1 + w//2, 0)
                    #   V chunk sc: read by ctx tasks popping ~one step later
                    intra = {}

                    def add(hp, qw, u, fn):
                        intra.setdefault((hp, qw, u), []).append(fn)

                    def addv(hp, qw, u, sc):
                        add(hp, qw, u, lambda: emit_v_chunk(sc))

                    def addqk(hp, qw, u, fo):
                        add(hp, qw, u, lambda: emit_qk_chain(fo))

                    def addq8(hp, qw, u, fo, w):
                        add(hp, qw, u, lambda: emit_qk8_chain(fo, w))

                    # --- hp 0 --- (all 16 V chunks live here; k8(3,*) and
                    # q8(0,*) feed hp0's own fp8 windows)
                    # Step order: plain nested (hp, qw); (2,3) runs
                    # last with the inline early-ctx finish.
                    steps = [(hp, qw) for hp in range(3)
                             for qw in range(QW)]
                    # Placement rule: an intra item at slot (hp, qw, u) is
                    # emitted AFTER scores of unit u+1 (emit-ahead pipeline),
                    # so it may only feed units >= u+2 of its own step.
                    addq8(0, 0, 0, 3, 0)
                    addq8(0, 0, 0, 0, 0)
                    addv(0, 0, 0, 0)
                    addq8(0, 0, 1, 3, 1)
                    addq8(0, 0, 1, 0, 1)
                    addv(0, 0, 1, 1)
                    addv(0, 1, 0, 2)
                    addq8(0, 1, 0, 3, 2)
                    addv(0, 1, 1, 3)
                    addq8(0, 1, 1, 3, 3)
                    addq8(0, 1, 1, 0, 2)
                    addq8(0, 1, 2, 0, 3)
                    addv(0, 1, 3, 4)
                    addq8(0, 1, 3, 3, 4)
                    addv(0, 2, 0, 5)
                    addq8(0, 2, 0, 3, 5)
                    addv(0, 2, 1, 6)
                    addq8(0, 2, 1, 0, 4)
                    addv(0, 2, 2, 7)
                    addq8(0, 2, 3, 0, 5)
                    addv(0, 2, 3, 8)
                    addq8(0, 2, 4, 3, 6)
                    addv(0, 2, 4, 9)
                    addq8(0, 2, 5, 3, 7)
                    addv(0, 3, 0, 10)
                    addv(0, 3, 1, 11)
                    addv(0, 3, 2, 12)
                    addv(0, 3, 3, 13)
                    addv(0, 3, 4, 14)
                    addqk(0, 3, 5, 1)
                    addv(0, 3, 5, 15)
                    addqk(0, 3, 6, 4)
                    addq8(0, 3, 6, 4, 0)
                    addq8(0, 3, 7, 1, 0)
                    addq8(1, 0, 0, 4, 1)
                    addq8(1, 0, 0, 1, 1)
                    addq8(1, 0, 1, 4, 2)
                    addq8(1, 1, 0, 4, 3)
                    addq8(1, 1, 1, 1, 2)
                    addq8(1, 1, 2, 1, 3)
                    addq8(1, 1, 3, 4, 4)
                    addq8(1, 2, 0, 4, 5)
                    addq8(1, 2, 1, 1, 4)
                    addq8(1, 2, 3, 1, 5)
                    addq8(1, 2, 4, 4, 6)
                    addq8(1, 2, 5, 4, 7)
                    addqk(1, 3, 1, 2)
                    addqk(1, 3, 2, 5)
                    addq8(1, 3, 3, 5, 0)
                    addq8(1, 3, 4, 5, 1)
                    addq8(1, 3, 5, 2, 0)
                    addq8(1, 3, 6, 2, 1)
                    addq8(1, 3, 7, 5, 2)
                    addq8(2, 0, 0, 5, 3)
                    addq8(2, 0, 1, 2, 2)
                    addq8(2, 1, 0, 5, 4)
                    addq8(2, 1, 1, 2, 3)
                    addq8(2, 1, 2, 5, 5)
                    addq8(2, 2, 0, 5, 6)
                    addq8(2, 2, 1, 2, 4)
                    addq8(2, 2, 3, 2, 5)
                    addq8(2, 2, 4, 5, 7)

                    for si, (hp, qw) in enumerate(steps):
                        nu = 2 * qw + 2
                        last = (si == len(steps) - 1)
                        Ets = []
                        get_psc, ctx_task = make_step(hp, qw, Ets)
                        if last:
                            # run all but the final two units normally (with
                            # harder draining), then emit the last two units'
                            # scores up front so ACT stays fed while the PE
                            # drains the backlog and runs the tail chains
                            Ets.append(emit_scores(hp, qw, 0))
                            for u in range(nu - 2):
                                if u + 1 < nu - 2:
                                    Ets.append(emit_scores(hp, qw, u + 1))
                                for fn in intra.get((hp, qw, u), ()):
                                    fn()
                                flush_one()
                            Ets.append(emit_scores(hp, qw, nu - 2))
                            Ets.append(emit_scores(hp, qw, nu - 1))
                            while work:
                                work.pop(0)()
                            st = {}
                            for qc in range(4):
                                ctx_task(qc)()
                            for qc in range(4):
                                emit_finish_qc(hp, qw, qc, get_psc(), st,
                                               outproj=False)
                            for qc in range(4):
                                emit_outproj_sc(4 * qw + qc, tail=True)
                            continue
                        # emit-ahead software pipeline                        # emit-ahead software pipeline: unit u+1's scores go
                        # out BEFORE unit u's intra chains / deferred pops, so
                        # the next exps are never queued behind filler work on
                        # the in-order PE stream.
                        Ets.append(emit_scores(hp, qw, 0))
                        for u in range(nu):
                            if u + 1 < nu:
                                Ets.append(emit_scores(hp, qw, u + 1))
                            for fn in intra.get((hp, qw, u), ()):
                                fn()
                            flush_one()
                        for qc in range(4):
                            work.append(ctx_task(qc))
                        finish_step(hp, qw, get_psc)
                    while work:
                        flush_one()

                # PE p-state warmup: dummy matmuls on a memset tile keep the
                # PE busy through the startup DMAs so the real startup chains
                # run at a higher clock (the cost model ramps 0.65 -> 1.2 ->
                # 2.4 GHz with continuous execution).
                warm_sb = PI.tile([128, 128], BF16, name="warm_sb")
                nc.gpsimd.memset(warm_sb[:], 0.5)
                wps = ps_tile([128, 512], "ppro_t", 2)
                for _ in range(24):
                    nc.tensor.matmul(wps[:, 0:128], warm_sb[:], warm_sb[:],
                                     start=True, stop=True,
                                     skip_group_check=True)

                # start-up: only what the first scores need
                emit_qk_chain(0)
                emit_qk_chain(3, on_act=True)
                emit_attention()

    return _patch_multiwait(nc)


_NC = {}


def _get_nc(with_bias=True):
    if with_bias not in _NC:
        _NC[with_bias] = build_nc(with_bias=with_bias)
    return _NC[with_bias]


def _prep_core_inputs(x, in_proj_w, in_proj_b, out_w, out_b):
    """Build the 8 per-core input dicts (host-side shard + transpose)."""
    import ml_dtypes
    bf16 = ml_dtypes.bfloat16
    fp8 = ml_dtypes.float8_e4m3
    # 0/1 keep-mask for S^T[k, q] diagonal blocks: keep where k <= q
    tri_np = (np.arange(128)[:, None] <= np.arange(128)[None, :])
    tri_bf = tri_np.astype(bf16)
    tri_f8 = tri_np.astype(fp8)
    id_bf = np.eye(128, dtype=np.float32).astype(bf16)
    ones_np = np.ones((1, 128), np.float32).astype(bf16)

    xT_by_b = [np.ascontiguousarray(np.asarray(x[b]).T) for b in range(B)]
    xT_bf = [a.astype(bf16) for a in xT_by_b]
    xT_f8 = [a.astype(fp8) for a in xT_by_b]

    in_maps = []
    for c in range(8):
        b = c // 2
        g = c % 2
        f0 = FPC * g
        Wq = np.asarray(in_proj_w[f0:f0 + FPC])
        Wk = np.asarray(in_proj_w[E + f0:E + f0 + FPC])
        Wv = np.asarray(in_proj_w[2 * E + f0:2 * E + f0 + FPC])
        bq = np.asarray(in_proj_b[f0:f0 + FPC])
        bk = np.asarray(in_proj_b[E + f0:E + f0 + FPC])
        bvv = np.asarray(in_proj_b[2 * E + f0:2 * E + f0 + FPC])
        Wo = np.asarray(out_w[:, f0:f0 + FPC])
        bqk_np = np.concatenate([bq, bk]).astype(np.float32).reshape(6, 128).T
        wqkT_np = np.ascontiguousarray(
            np.concatenate([Wq, Wk], axis=0).T).astype(np.float32)
        in_maps.append({
            "xT": xT_bf[b],
            "xT8": xT_f8[b],
            "wqkT": wqkT_np.astype(bf16),
            "wqkT8": wqkT_np.astype(fp8),
            "wvT": np.ascontiguousarray(Wv.T).astype(bf16),
            "woT": np.ascontiguousarray(Wo.T).astype(bf16),
            "bqk": np.ascontiguousarray(bqk_np),
            "bv": bvv.reshape(1, FPC).astype(bf16),
            # out bias only on even cores so the host-side pair-sum is exact
            "bo": np.asarray(out_b).reshape(1, E).astype(bf16) if g == 0
                  else np.zeros((1, E), bf16),
            "tri": tri_bf,
            "tri8": tri_f8,
            "ident": id_bf,
            "ones": ones_np,
        })
    return in_maps


def kernel(x, in_proj_w, in_proj_b, out_w, out_b):
    zero_bias = (not np.any(np.asarray(in_proj_b))) and \
                (not np.any(np.asarray(out_b)))
    nc = _get_nc(with_bias=not zero_bias)
    in_maps = _prep_core_inputs(x, in_proj_w, in_proj_b, out_w, out_b)
    res = run_bass_kernel_spmd(nc, in_maps, core_ids=list(range(8)))
    out = np.empty((B, S, E), np.float32)
    for b in range(B):
        out[b] = res.results[2 * b]["y"] + res.results[2 * b + 1]["y"]
    return out


# revision 43
# speedup vs baseline: 1.0310x; 1.0076x over previous
"""Multi-head attention (B=4, S=2048, E=768, H=12, D=64, causal) on 8 trn2
NeuronCores.

Sharding: core c -> batch b = c//2, head-half g = c%2 (6 heads each).
Each core computes its 6 heads' attention plus the partial output
projection; the host sums the two half-head partials per batch.

Hybrid precision (validated against the fp32 reference in numpy):
  - Rows q < 512 (q-window 0) have concentrated softmax (few keys), so
    their whole attention path stays bf16: bf16 qk-projection (only the
    first 512 q/k columns are needed causally), bf16 scores, bf16 exp'd
    weights, bf16 ctx with bf16 V.
  - Rows q >= 512 average fp8 quantization noise over many keys: fp8e4
    q/k (from an fp8 DoubleRow qk-projection over e-pairs), plain-fp8
    score matmuls, exp'd weights written fp8e4 by ACT (bias -2.5 keeps
    them in fp8 range; softmax normalization cancels the shift), and the
    ctx matmul runs fp8 DoubleRow over k-chunk pairs (half the
    instructions, quarter the PE cycles of bf16).
  Combined max-rel-err ~8e-3 vs the 2e-2 gate.

On-device strategy (per core) otherwise matches the bf16 design:
  - Host pre-transposes x / weight slices so every contraction dim lands
    on SBUF partitions; x ships in bf16 and fp8.
  - Scores are computed TRANSPOSED (S^T[k, q] = K^T Q); diagonal tiles
    shrink their moving window; in-diagonal-block causal masking is a
    DVE multiply by a 0/1 lower-triangle.
  - V_aug packs a ones column per head so the PE emits softmax row-sums
    for free; V is stored bf16 (k-chunks 0-3, for the bf16 window) and
    fp8 in per-unit pair layout [128, 2, 65*HPC] for DoubleRow.
  - ctx lands [q, d+1] with q on partitions; normalization is a DVE
    reciprocal + per-qc scalar multiply; ONE merged [128,128] PE
    transpose per (hp, qw, qc) brings both heads' ctx^T back.
  - The attention phase is ACT(exp)-throughput-bound; projections and
    ctx/norm/out-proj run as deferred tasks interleaved between units.
"""
import sys, json, os

for _p in ("/opt/trn_rl_repo",):
    if _p not in sys.path and os.path.isdir(_p):
        sys.path.insert(0, _p)

import numpy as np
import concourse.bass as bass
import concourse.mybir as mybir
import concourse.tile as tile
from concourse.bass_utils import run_bass_kernel_spmd

B, S, E, H, D = 4, 2048, 768, 12, 64
HPC = H // 2          # heads per core = 6
FPC = HPC * D         # features per core per q/k/v = 384
EC = E // 128         # 6 contraction chunks for projections
SC = S // 128         # 16 s-chunks
QW = S // 512         # 4 q-windows
KC = S // 128         # 16 k-chunks
F32 = mybir.dt.float32
BF16 = mybir.dt.bfloat16
FP8 = mybir.dt.float8e4
EXP = mybir.ActivationFunctionType.Exp
DR = mybir.MatmulPerfMode.DoubleRow
EBIAS = -2.5          # exp bias for fp8 windows (softmax-invariant)


def _patch_multiwait(nc, max_waits=1):
    """This container's walrus rejects instructions with more than one sync
    wait. Split excess waits onto same-engine NOPs emitted immediately
    before the instruction (same-engine streams are order-preserving)."""
    raw = nc.to_json_bytes()
    m = json.loads(raw)
    for f in m["functions"]:
        for b in f["blocks"]:
            out = []
            for inst in b["instructions"]:
                si = inst.get("sync_info") or {}
                ws = si.get("on_wait") or []
                if len(ws) > max_waits:
                    eng = inst["engine"]
                    for i, w in enumerate(ws[:-max_waits]):
                        out.append({
                            "debug": inst.get("debug", 0), "engine": eng,
                            "ins": [], "name": inst["name"] + f"-mw{i}",
                            "opcode": "NoOp", "outs": [],
                            "sync_info": {"on_update": [], "on_wait": [w]},
                        })
                    si["on_wait"] = ws[-max_waits:]
                out.append(inst)
            b["instructions"] = out
    patched = json.dumps(m).encode()
    nc.to_json_bytes = lambda: patched
    return nc


def build_nc(with_bias=True):
    nc = bass.Bass()
    xT = nc.dram_tensor("xT", [E, S], BF16, kind="ExternalInput")
    xT8 = nc.dram_tensor("xT8", [E, S], FP8, kind="ExternalInput")
    wqkT = nc.dram_tensor("wqkT", [E, 2 * FPC], BF16, kind="ExternalInput")
    # startup slice: f-chunks 0 and 3 (cols 0:128 and 384:512) packed so the
    # two startup chains need ONE DMA (each DMA costs ~1.7us serial
    # HWDGE+transfer time at the head of the pipeline)
    wqkS = nc.dram_tensor("wqkS", [E, 256], BF16, kind="ExternalInput")
    wqkT8 = nc.dram_tensor("wqkT8", [E, 2 * FPC], FP8, kind="ExternalInput")
    wvT = nc.dram_tensor("wvT", [E, FPC], BF16, kind="ExternalInput")
    woT = nc.dram_tensor("woT", [FPC, E], BF16, kind="ExternalInput")
    bqk = nc.dram_tensor("bqk", [128, 2 * FPC // 128], F32, kind="ExternalInput")
    bv = nc.dram_tensor("bv", [1, FPC], BF16, kind="ExternalInput")
    bo = nc.dram_tensor("bo", [1, E], BF16, kind="ExternalInput")
    tri = nc.dram_tensor("tri", [128, 128], BF16, kind="ExternalInput")
    tri8 = nc.dram_tensor("tri8", [128, 128], FP8, kind="ExternalInput")
    ident = nc.dram_tensor("ident", [128, 128], BF16, kind="ExternalInput")
    ones = nc.dram_tensor("ones", [1, 128], BF16, kind="ExternalInput")
    y = nc.dram_tensor("y", [S, E], F32, kind="ExternalOutput")

    with tile.TileContext(nc) as tc, \
         nc.allow_low_precision(reason="hybrid bf16/fp8 pipeline by design"):
        with tc.tile_pool(name="persist", bufs=1) as P, \
             tc.tile_pool(name="ps", bufs=1, space="PSUM") as PS:
            # --- persistent tiles (bottom-of-stack, live whole kernel)
            # bf16 q/k: only columns [0, 512) are ever read (q-window 0)
            qkT_sb = [P.tile([128, 512], BF16, name=f"qkT{i}") for i in range(6)]
            # fp8 q/k: q chunks hold cols [512, 2048), k chunks [0, 2048)
            qkT8_sb = [P.tile([128, S], FP8, name=f"qkT8_{i}") for i in range(6)]
            # bf16 V for the bf16 window's ctx (k-chunks 0..3 only)
            V_sb = [P.tile([128, 65 * HPC], BF16, name=f"V{i}") for i in range(4)]
            # fp8 V in unit-pair layout: per partition [t(2), h(6), 65]
            V8_sb = [P.tile([128, 2 * 65 * HPC], FP8, name=f"V8_{i}")
                     for i in range(KC // 2)]
            ctxT_sb = [P.tile([128, S], BF16, name=f"ctxT{i}") for i in range(3)]
            woT_sb = [P.tile([128, E], BF16, name=f"woT{i}") for i in range(3)]
            bqk_sb = P.tile([128, 6], F32, name="bqk_sb")
            bv_sb = P.tile([1, FPC], BF16, name="bv_sb")
            bo_sb = P.tile([1, E], BF16, name="bo_sb")
            tri_sb = P.tile([128, 128], BF16, name="tri_sb")
            tri8_sb = P.tile([128, 128], FP8, name="tri8_sb")
            id_sb = P.tile([128, 128], BF16, name="id_sb")
            on_sb = P.tile([1, 128], BF16, name="on_sb")
            ebias_sb = P.tile([128, 1], F32, name="ebias_sb")
            nc.gpsimd.memset(ebias_sb[:], EBIAS)

            def ps_tile(shape, tag, bufs, dtype=F32):
                return PS.tile(shape, dtype, name=tag, tag=tag, bufs=bufs)

            # ============ phase 1 (projections) + attention, interleaved ====
            with tc.tile_pool(name="inp", bufs=1) as PI, \
                 tc.tile_pool(name="esb", bufs=14) as EP, \
                 tc.tile_pool(name="nrm", bufs=12) as NP, \
                 tc.tile_pool(name="osb", bufs=3) as OP:
                # consolidated input tiles; one DMA dispatch per tensor region
                xT_sb = PI.tile([128, EC * S], BF16, name="xT_all")
                xT8_sb = PI.tile([128, EC * S], FP8, name="xT8_all")
                wqkT_sb = PI.tile([128, EC * 2 * FPC], BF16, name="wqkT_all")
                wqkT8_sb = PI.tile([128, EC * 2 * FPC], FP8, name="wqkT8_all")
                wvT_sb = PI.tile([128, EC * FPC], BF16, name="wvT_all")
                wqkS_sb = PI.tile([128, EC * 256], BF16, name="wqkS_all")
                xs = xT_sb[:].rearrange("p (e s) -> p e s", e=EC)
                xd = xT.ap().rearrange("(e p) s -> p e s", p=128)
                x8s = xT8_sb[:].rearrange("p (e s) -> p e s", e=EC)
                x8d = xT8.ap().rearrange("(e p) s -> p e s", p=128)
                qs = wqkT_sb[:].rearrange("p (e f) -> p e f", e=EC)
                qd = wqkT.ap().rearrange("(e p) f -> p e f", p=128)
                q8s = wqkT8_sb[:].rearrange("p (e f) -> p e f", e=EC)
                q8d = wqkT8.ap().rearrange("(e p) f -> p e f", p=128)
                # DMA order: startup chains first (bf16 wqkT fo 0/3, then xT
                # cols 0:512 per e-chunk so the startup chains pipeline with
                # the DMA stream), then the rest in rough order of first use.
                nc.sync.dma_start(
                    wqkS_sb[:].rearrange("p (e f) -> p e f", e=EC),
                    wqkS.ap().rearrange("(e p) f -> p e f", p=128))
                nc.sync.dma_start(xs[:, 0:2, 0:512], xd[:, 0:2, 0:512])
                nc.sync.dma_start(xs[:, 2:6, 0:512], xd[:, 2:6, 0:512])
                nc.sync.dma_start(tri_sb[:], tri.ap())
                nc.sync.dma_start(q8s[:], q8d[:])
                nc.sync.dma_start(x8s[:, :, 0:1024], x8d[:, :, 0:1024])
                nc.sync.dma_start(
                    wvT_sb[:].rearrange("p (e f) -> p e f", e=EC),
                    wvT.ap().rearrange("(e p) f -> p e f", p=128))
                nc.sync.dma_start(tri8_sb[:], tri8.ap())
                nc.sync.dma_start(xs[:, :, 512:S], xd[:, :, 512:S])
                nc.sync.dma_start(x8s[:, :, 1024:S], x8d[:, :, 1024:S])
                nc.sync.dma_start(qs[:, :, 128:384], qd[:, :, 128:384])
                nc.sync.dma_start(qs[:, :, 512:768], qd[:, :, 512:768])
                nc.sync.dma_start(id_sb[:], ident.ap())
                for i in range(3):
                    nc.sync.dma_start(woT_sb[i][:],
                                      woT.ap()[128 * i:128 * (i + 1), :])
                nc.sync.dma_start(bqk_sb[:], bqk.ap())
                nc.sync.dma_start(bv_sb[:], bv.ap())
                nc.sync.dma_start(on_sb[:], ones.ap())
                nc.sync.dma_start(bo_sb[:], bo.ap())

                def emit_qk_chain(fo, on_act=False, cols=(0, 512)):
                    """bf16 qk-proj for f-chunk fo, s-cols [0, 512) only
                    (optionally a sub-range, for the startup k-chain).
                    f-chunks 0/3 read the packed one-DMA startup tile."""
                    c0, c1 = cols
                    pair = ps_tile([128, 512], "ppro_t", 2)
                    for ecc in range(EC):
                        if fo in (0, 3):
                            stat = wqkS_sb[:, 256 * ecc + 128 * (fo // 3):
                                           256 * ecc + 128 * (fo // 3 + 1)]
                        else:
                            stat = wqkT_sb[:, 768 * ecc + 128 * fo:
                                           768 * ecc + 128 * (fo + 1)]
                        nc.tensor.matmul(
                            pair[:, c0:c1],
                            stat,
                            xT_sb[:, S * ecc + c0:S * ecc + c1],
                            start=(ecc == 0), stop=(ecc == EC - 1),
                            skip_group_check=True)
                    dst = qkT_sb[fo][:, c0:c1]
                    src = pair[:, c0:c1]
                    if with_bias:
                        nc.vector.tensor_scalar_add(
                            dst, src, bqk_sb[:, fo:fo + 1])
                    elif on_act:
                        nc.scalar.copy(dst, src)
                    else:
                        nc.vector.tensor_copy(dst, src)

                def emit_qk8_chain(fo, win):
                    """fp8 DoubleRow qk-proj for f-chunk fo, one 256-col
                    window. q chunks (fo<3): win 0..5 -> cols 512+256*win.
                    k chunks (fo>=3): win 0..7 -> cols 256*win."""
                    c0 = (512 if fo < 3 else 0) + 256 * win
                    pair = ps_tile([128, 512], "ppro_t", 2)
                    wv = wqkT8_sb[:].rearrange("p (e f) -> p e f", e=EC)
                    xv = xT8_sb[:].rearrange("p (e s) -> p e s", e=EC)
                    for ecp in range(EC // 2):
                        nc.tensor.matmul(
                            pair[:, 0:256],
                            wv[:, 2 * ecp:2 * ecp + 2,
                               128 * fo:128 * (fo + 1)],
                            xv[:, 2 * ecp:2 * ecp + 2, c0:c0 + 256],
                            start=(ecp == 0), stop=(ecp == EC // 2 - 1),
                            skip_group_check=True, perf_mode=DR)
                    dst = qkT8_sb[fo][:, c0:c0 + 256]
                    src = pair[:, 0:256]
                    if with_bias:
                        nc.vector.tensor_scalar_add(
                            dst, src, bqk_sb[:, fo:fo + 1])
                    else:
                        nc.vector.tensor_copy(dst, src)

                def emit_v_chunk(sc):
                    """V-proj for s-chunk sc (bf16), copied out bf16 (sc<4,
                    for the bf16 window) and fp8 (unit-pair layout, always)."""
                    psv = ps_tile([128, FPC], "ppro_t", 2)
                    for ecc in range(EC):
                        nc.tensor.matmul(
                            psv[:],
                            xT_sb[:, S * ecc + 128 * sc:
                                  S * ecc + 128 * (sc + 1)],
                            wvT_sb[:, FPC * ecc:FPC * (ecc + 1)],
                            start=(ecc == 0),
                            stop=(not with_bias and ecc == EC - 1),
                            skip_group_check=True)
                    if with_bias:
                        nc.tensor.matmul(psv[:], on_sb[:, 0:128],
                                         bv_sb[:], start=False, stop=True,
                                         skip_group_check=True)
                    u, t = divmod(sc, 2)
                    v8 = V8_sb[u][:].rearrange("p (t h x) -> p t h x",
                                               t=2, x=65)
                    nc.vector.tensor_copy(
                        v8[:, t, :, 0:64],
                        psv[:].rearrange("p (h x) -> p h x", x=64))
                    nc.gpsimd.memset(v8[:, t, :, 64:65], 1.0)
                    if sc < 4:
                        vv = V_sb[sc][:].rearrange("p (h x) -> p h x", x=65)
                        nc.vector.tensor_copy(
                            vv[:, :, 0:64],
                            psv[:].rearrange("p (h x) -> p h x", x=64))
                        nc.gpsimd.memset(vv[:, :, 64:65], 1.0)

                def emit_scores(hp, qw, u):
                    """Scores S^T[k, q] for a pair of k-chunks, both heads,
                    + exp (+ DVE causal masks on diag blocks). qw=0 runs
                    bf16 (writes Et bf16); qw>=1 runs fp8 (writes Et fp8,
                    exp bias EBIAS). Returns Et [128, 2048]
                    (cols 1024*hd + 512*half + qlocal)."""
                    fp8 = qw >= 1
                    if fp8:
                        qT, kT = qkT8_sb[hp], qkT8_sb[3 + hp]
                        qcol = 512 * qw
                        trit = tri8_sb
                        edt = FP8
                    else:
                        qT, kT = qkT_sb[hp], qkT_sb[3 + hp]
                        qcol = 0       # bf16 tiles hold only cols [0,512)
                        trit = tri_sb
                        edt = BF16
                    pss = {hd: ps_tile([128, 1024], "pss_t", 2)
                           for hd in range(2)}
                    Et = EP.tile([128, 2048], edt, name="E_t")
                    for half in range(2):
                        ki = 2 * u + half
                        j = ki - 4 * qw
                        c = 128 * j if j > 0 else 0
                        # strict row-group alternation (base 0,64,0,64) so
                        # score matmul pairs run on separate PE row groups;
                        # diag tiles shrink the moving window
                        for hd in range(2):
                            base = 64 * hd
                            nc.tensor.matmul(
                                pss[hd][:, 512 * half + c:512 * (half + 1)],
                                kT[base:base + 64, 128 * ki:128 * (ki + 1)],
                                qT[base:base + 64,
                                   qcol + c:qcol + 512],
                                start=True, stop=True,
                                skip_group_check=True)
                    j0 = 2 * u - 4 * qw
                    j1 = j0 + 1
                    c0 = 128 * j0 if j0 > 0 else 0
                    bias = ebias_sb[:, 0:1] if fp8 else 0.0
                    # one exp per head spans both halves when contiguous;
                    # when the half-1 diag shrink leaves an unwritten PSUM
                    # gap, split the exp around it -- EXCEPT in fp8 windows
                    # with a single-block gap (j1 == 1): there one merged exp
                    # over the gap is cheaper than a second instruction. The
                    # gap columns hold stale-but-bounded old scores (the slot
                    # was written by earlier units), their exp is finite, and
                    # the masked-out block is never read by any ctx matmul.
                    for hd in range(2):
                        if fp8 and j1 == 1:
                            nc.scalar.activation(
                                Et[:, 1024 * hd:1024 * (hd + 1)],
                                pss[hd][:, 0:1024], EXP, scale=0.125,
                                bias=bias)
                        elif j1 > 0:
                            nc.scalar.activation(
                                Et[:, 1024 * hd + c0:1024 * hd + 512],
                                pss[hd][:, c0:512], EXP, scale=0.125,
                                bias=bias)
                            c1 = 128 * j1
                            nc.scalar.activation(
                                Et[:, 1024 * hd + 512 + c1:1024 * (hd + 1)],
                                pss[hd][:, 512 + c1:1024], EXP, scale=0.125,
                                bias=bias)
                        else:
                            nc.scalar.activation(
                                Et[:, 1024 * hd + c0:1024 * (hd + 1)],
                                pss[hd][:, c0:1024], EXP, scale=0.125,
                                bias=bias)
                    # causal mask inside the diagonal 128x128 blocks
                    for half in range(2):
                        j = 2 * u + half - 4 * qw
                        if j >= 0:
                            for hd in range(2):
                                off = 1024 * hd + 512 * half + 128 * j
                                nc.vector.tensor_mul(
                                    Et[:, off:off + 128],
                                    Et[:, off:off + 128], trit[:])
                    return Et

                def emit_ctx_qc(hp, qw, qc, Ets, psc):
                    """ctx for one q-chunk, both heads. qw=0: bf16 per-ki
                    matmuls with V_sb. qw>=1: fp8 DoubleRow over k-chunk
                    pairs (units) with V8_sb, plus a plain-fp8 tail when
                    the diagonal cuts a unit in half."""
                    nk = 4 * qw + qc + 1        # k-chunks 0..4qw+qc
                    if qw == 0:
                        for ki in range(nk):
                            u, half = divmod(ki, 2)
                            Et = Ets[u]
                            for hd in range(2):
                                h = 2 * hp + hd
                                nc.tensor.matmul(
                                    psc[hd][:, 65 * qc:65 * qc + 65],
                                    Et[:, 1024 * hd + 512 * half + 128 * qc:
                                        1024 * hd + 512 * half + 128 * qc
                                        + 128],
                                    V_sb[ki][:, 65 * h:65 * h + 65],
                                    start=(ki == 0), stop=(ki == nk - 1),
                                    skip_group_check=True)
                        return
                    nu_full = nk // 2           # full DR unit-pairs
                    tail = nk % 2               # lone half-0 chunk at the end
                    for hd in range(2):
                        h = 2 * hp + hd
                        for u in range(nu_full):
                            ev = Ets[u][:, 1024 * hd:1024 * (hd + 1)] \
                                .rearrange("p (t q) -> p t q", t=2)
                            v8 = V8_sb[u][:].rearrange(
                                "p (t h x) -> p t h x", t=2, x=65)
                            nc.tensor.matmul(
                                psc[hd][:, 65 * qc:65 * qc + 65],
                                ev[:, :, 128 * qc:128 * qc + 128],
                                v8[:, :, h, :],
                                start=(u == 0),
                                stop=(tail == 0 and u == nu_full - 1),
                                skip_group_check=True, perf_mode=DR)
                        if tail:
                            u = nu_full
                            v8 = V8_sb[u][:].rearrange(
                                "p (t h x) -> p t h x", t=2, x=65)
                            nc.tensor.matmul(
                                psc[hd][:, 65 * qc:65 * qc + 65],
                                Ets[u][:, 1024 * hd + 128 * qc:
                                       1024 * hd + 128 * qc + 128],
                                v8[:, 0, h, :],
                                start=(nu_full == 0), stop=True,
                                skip_group_check=True)

                def emit_finish_qc(hp, qw, qc, psc, st, outproj=True):
                    """Per-qc finish for the final step: craw slice + recip +
                    normalize + transpose + ctxT copy + out-proj, so each
                    s-chunk's chain starts the moment its own ctx is done."""
                    if "ctxn" not in st:
                        st["ctxn"] = NP.tile([128, 512], BF16, name="ctxn_t")
                    # psc_t's 2 slots are BOTH live (the psc accumulators)
                    # until the last craw is copied out, so pt must come from
                    # the pss_t ring (scores are done with it by now).
                    pt = ps_tile([128, 512], "pss_t", 2, dtype=BF16)
                    ctxn = st["ctxn"]
                    for hd in range(2):
                        craw = NP.tile([128, 65], F32, name="crawq_t")
                        nc.vector.tensor_copy(
                            craw[:], psc[hd][:, 65 * qc:65 * qc + 65])
                        rinv = NP.tile([128, 1], F32, name="rinvq_t")
                        nc.vector.reciprocal(
                            rinv[:], craw[:, 64:65])
                        nc.vector.tensor_scalar_mul(
                            ctxn[:, 128 * qc + 64 * hd:
                                 128 * qc + 64 * (hd + 1)],
                            craw[:, 0:64], rinv[:])
                    nc.tensor.transpose(
                        pt[:, 128 * qc:128 * (qc + 1)],
                        ctxn[:, 128 * qc:128 * (qc + 1)], id_sb[:])
                    nc.scalar.copy(
                        ctxT_sb[hp][:, 512 * qw + 128 * qc:
                                     512 * qw + 128 * (qc + 1)],
                        pt[:, 128 * qc:128 * (qc + 1)])
                    if outproj:
                        emit_outproj_sc(4 * qw + qc, tail=True)

                def emit_norm_head(hp, qw, psc, st):
                    """Copy raw ctx out of PSUM (freeing psc) and compute
                    reciprocal row-sums."""
                    craws = []
                    for hd in range(2):
                        craw = NP.tile([128, 260], F32, name="craw_t")
                        nc.vector.tensor_copy(craw[:], psc[hd][:])
                        craws.append(craw)
                    pt = ps_tile([128, 512], "psc_t", 2, dtype=BF16)
                    ctxn = NP.tile([128, 512], BF16, name="ctxn_t")
                    rinvs = []
                    for hd in range(2):
                        craw = craws[hd]
                        cv = craw[:].rearrange("p (q x) -> p q x", x=65)
                        rinv = NP.tile([128, 4], F32, name="rinv_t")
                        nc.vector.reciprocal(
                            rinv[:].rearrange("p (q x) -> p q x", x=1),
                            cv[:, :, 64:65])
                        rinvs.append((craw, rinv))
                    st["pt"] = pt
                    st["ctxn"] = ctxn
                    st["rinvs"] = rinvs

                def emit_norm_qc(hp, qw, qc, st):
                    """Normalize + ONE merged 2-head transpose + copy out one
                    128-column ctxT block."""
                    pt, ctxn, rinvs = st["pt"], st["ctxn"], st["rinvs"]
                    for hd in range(2):
                        craw, rinv = rinvs[hd]
                        nc.vector.tensor_scalar_mul(
                            ctxn[:, 128 * qc + 64 * hd:
                                 128 * qc + 64 * (hd + 1)],
                            craw[:, 65 * qc:65 * qc + 64],
                            rinv[:, qc:qc + 1])
                    nc.tensor.transpose(
                        pt[:, 128 * qc:128 * (qc + 1)],
                        ctxn[:, 128 * qc:128 * (qc + 1)], id_sb[:])
                    nc.vector.tensor_copy(
                        ctxT_sb[hp][:, 512 * qw + 128 * qc:
                                     512 * qw + 128 * (qc + 1)],
                        pt[:, 128 * qc:128 * (qc + 1)])

                def emit_outproj_sc(sc, tail=False, ring=None):
                    osb = OP.tile([128, E], F32, name="osb_t")
                    # tail chains run while psc_t's two slots are still live
                    # (the psc accumulators), so they must use pss_t;
                    # explicit ring= alternates banks for back-to-back chains
                    tg, nb = (ring, 2) if ring else \
                        (("pss_t", 2) if tail else ("psc_t", 2))
                    pos = {0: ps_tile([128, 512], tg, nb),
                           512: ps_tile([128, 256], tg, nb)}
                    for c in range(3):
                        for f0, fn in ((0, 512), (512, 256)):
                            nc.tensor.matmul(
                                pos[f0][:, 0:fn],
                                ctxT_sb[c][:, 128 * sc:128 * (sc + 1)],
                                woT_sb[c][:, f0:f0 + fn],
                                start=(c == 0),
                                stop=(not with_bias and c == 2),
                                skip_group_check=True)
                    for f0, fn in ((0, 512), (512, 256)):
                        if with_bias:
                            nc.tensor.matmul(pos[f0][:, 0:fn],
                                             on_sb[:, 0:128],
                                             bo_sb[:, f0:f0 + fn],
                                             start=False, stop=True,
                                             skip_group_check=True)
                        # in the tail ACT is idle (all exps done): put the
                        # copy-outs there and DMA each f-window as it lands
                        if tail:
                            nc.scalar.copy(osb[:, f0:f0 + fn],
                                           pos[f0][:, 0:fn])
                            nc.sync.dma_start(
                                y.ap()[128 * sc:128 * (sc + 1), f0:f0 + fn],
                                osb[:, f0:f0 + fn])
                        else:
                            nc.vector.tensor_copy(osb[:, f0:f0 + fn],
                                                  pos[f0][:, 0:fn])
                    if not tail:
                        nc.sync.dma_start(y.ap()[128 * sc:128 * (sc + 1), :],
                                          osb[:])

                def emit_attention():
                    # software pipeline: ctx runs as per-q-chunk deferred
                    # tasks queued when a step's scores complete; one task
                    # pops per unit so ctx/norm/out-proj spread between
                    # later units while ACT chews on exps.
                    work = []

                    def flush_one():
                        if work:
                            work.pop(0)()
                        if len(work) > 3:
                            work.pop(0)()

                    def make_step(hp, qw, Ets):
                        holder = {}

                        def get_psc():
                            if not holder:
                                holder[0] = {
                                    hd: ps_tile([128, 260], "psc_t", 2)
                                    for hd in range(2)}
                            return holder[0]

                        def ctx_task(qc):
                            return lambda: emit_ctx_qc(
                                hp, qw, qc, Ets, get_psc())
                        return get_psc, ctx_task

                    def finish_step(hp, qw, get_psc):
                        st = {}
                        if hp < 2:
                            def norm_all():
                                emit_norm_head(hp, qw, get_psc(), st)
                                for qc in range(4):
                                    emit_norm_qc(hp, qw, qc, st)
                            work.append(norm_all)
                        else:
                            work.append(lambda: emit_norm_head(
                                hp, qw, get_psc(), st))

                            def norm_op(qc):
                                emit_norm_qc(hp, qw, qc, st)
                                emit_outproj_sc(4 * qw + qc)
                            for qc in range(4):
                                work.append(lambda qc=qc: norm_op(qc))

                    # phase-1 chains interleaved between attention units:
                    # (hp, qw, u) -> thunks emitted right after that unit's
                    # scores+flush. Deadlines:
                    #   bf16 qk chain fo in {hp', 3+hp'} before (hp', 0, 0)
                    #   fp8 k chain (3+hp', win w) before (hp', 1 + w//4,
                    #     u = w % ...) - k-chunks 2w,2w+1 first read at
                    #     (hp', qw >= 1, u = w)
                    #   fp8 q chain (hp', win w) before (hp', 1 + w//2, 0)
                    #   V chunk sc: read by ctx tasks popping ~one step later
                    intra = {}

                    def add(hp, qw, u, fn):
                        intra.setdefault((hp, qw, u), []).append(fn)

                    def addv(hp, qw, u, sc):
                        add(hp, qw, u, lambda: emit_v_chunk(sc))

                    def addqk(hp, qw, u, fo):
                        add(hp, qw, u, lambda: emit_qk_chain(fo))

                    def addq8(hp, qw, u, fo, w):
                        add(hp, qw, u, lambda: emit_qk8_chain(fo, w))

                    # --- hp 0 --- (all 16 V chunks live here; k8(3,*) and
                    # q8(0,*) feed hp0's own fp8 windows)
                    # Step order: plain nested (hp, qw); (2,3) runs
                    # last with the inline early-ctx finish.
                    steps = [(hp, qw) for hp in range(3)
                             for qw in range(QW)]
                    # Placement rule: an intra item at slot (hp, qw, u) is
                    # emitted AFTER scores of unit u+1 (emit-ahead pipeline),
                    # so it may only feed units >= u+2 of its own step.
                    addq8(0, 0, 0, 3, 0)
                    addq8(0, 0, 0, 0, 0)
                    addv(0, 0, 0, 0)
                    addq8(0, 0, 1, 3, 1)
                    addq8(0, 0, 1, 0, 1)
                    addv(0, 0, 1, 1)
                    addv(0, 1, 0, 2)
                    addq8(0, 1, 0, 3, 2)
                    addv(0, 1, 1, 3)
                    addq8(0, 1, 1, 3, 3)
                    addq8(0, 1, 1, 0, 2)
                    addq8(0, 1, 2, 0, 3)
                    addv(0, 1, 3, 4)
                    addq8(0, 1, 3, 3, 4)
                    addv(0, 2, 0, 5)
                    addq8(0, 2, 0, 3, 5)
                    addv(0, 2, 1, 6)
                    addq8(0, 2, 1, 0, 4)
                    addv(0, 2, 2, 7)
                    addq8(0, 2, 3, 0, 5)
                    addv(0, 2, 3, 8)
                    addq8(0, 2, 4, 3, 6)
                    addv(0, 2, 4, 9)
                    addq8(0, 2, 5, 3, 7)
                    addv(0, 3, 0, 10)
                    addv(0, 3, 1, 11)
                    addv(0, 3, 2, 12)
                    addv(0, 3, 3, 13)
                    addv(0, 3, 4, 14)
                    addqk(0, 3, 5, 1)
                    addv(0, 3, 5, 15)
                    addqk(0, 3, 6, 4)
                    addq8(0, 3, 6, 4, 0)
                    addq8(0, 3, 7, 1, 0)
                    addq8(1, 0, 0, 4, 1)
                    addq8(1, 0, 0, 1, 1)
                    addq8(1, 0, 1, 4, 2)
                    addq8(1, 1, 0, 4, 3)
                    addq8(1, 1, 1, 1, 2)
                    addq8(1, 1, 2, 1, 3)
                    addq8(1, 1, 3, 4, 4)
                    addq8(1, 2, 0, 4, 5)
                    addq8(1, 2, 1, 1, 4)
                    addq8(1, 2, 3, 1, 5)
                    addq8(1, 2, 4, 4, 6)
                    addq8(1, 2, 5, 4, 7)
                    addqk(1, 3, 1, 2)
                    addqk(1, 3, 2, 5)
                    addq8(1, 3, 3, 5, 0)
                    addq8(1, 3, 4, 5, 1)
                    addq8(1, 3, 5, 2, 0)
                    addq8(1, 3, 6, 2, 1)
                    addq8(1, 3, 7, 5, 2)
                    addq8(2, 0, 0, 5, 3)
                    addq8(2, 0, 1, 2, 2)
                    addq8(2, 1, 0, 5, 4)
                    addq8(2, 1, 1, 2, 3)
                    addq8(2, 1, 2, 5, 5)
                    addq8(2, 2, 0, 5, 6)
                    addq8(2, 2, 1, 2, 4)
                    addq8(2, 2, 3, 2, 5)
                    addq8(2, 2, 4, 5, 7)

                    for si, (hp, qw) in enumerate(steps):
                        nu = 2 * qw + 2
                        last = (si == len(steps) - 1)
                        Ets = []
                        get_psc, ctx_task = make_step(hp, qw, Ets)
                        if last:
                            # run all but the final two units normally (with
                            # harder draining), then emit the last two units'
                            # scores up front so ACT stays fed while the PE
                            # drains the backlog and runs the tail chains
                            Ets.append(emit_scores(hp, qw, 0))
                            for u in range(nu - 2):
                                if u + 1 < nu - 2:
                                    Ets.append(emit_scores(hp, qw, u + 1))
                                for fn in intra.get((hp, qw, u), ()):
                                    fn()
                                flush_one()
                            Ets.append(emit_scores(hp, qw, nu - 2))
                            Ets.append(emit_scores(hp, qw, nu - 1))
                            while work:
                                work.pop(0)()
                            st = {}
                            for qc in range(4):
                                ctx_task(qc)()
                            for qc in range(4):
                                emit_finish_qc(hp, qw, qc, get_psc(), st,
                                               outproj=False)
                            for qc in range(4):
                                emit_outproj_sc(4 * qw + qc, tail=True)
                            continue
                        # emit-ahead software pipeline                        # emit-ahead software pipeline: unit u+1's scores go
                        # out BEFORE unit u's intra chains / deferred pops, so
                        # the next exps are never queued behind filler work on
                        # the in-order PE stream.
                        Ets.append(emit_scores(hp, qw, 0))
                        for u in range(nu):
                            if u + 1 < nu:
                                Ets.append(emit_scores(hp, qw, u + 1))
                            for fn in intra.get((hp, qw, u), ()):
                                fn()
                            flush_one()
                        for qc in range(4):
                            work.append(ctx_task(qc))
                        finish_step(hp, qw, get_psc)
                    while work:
                        flush_one()

                # PE p-state warmup: dummy matmuls on a memset tile keep the
                # PE busy through the startup DMAs so the real startup chains
                # run at a higher clock (the cost model ramps 0.65 -> 1.2 ->
                # 2.4 GHz with continuous execution).
                warm_sb = PI.tile([128, 128], BF16, name="warm_sb")
                nc.gpsimd.memset(warm_sb[:], 0.5)
                wps = ps_tile([128, 512], "ppro_t", 2)
                for _ in range(34):
                    nc.tensor.matmul(wps[:, 0:128], warm_sb[:], warm_sb[:],
                                     start=True, stop=True,
                                     skip_group_check=True)

                # start-up: only what the first scores need
                emit_qk_chain(0)
                emit_qk_chain(3, on_act=True)
                emit_attention()

    return _patch_multiwait(nc)


_NC = {}


def _get_nc(with_bias=True):
    if with_bias not in _NC:
        _NC[with_bias] = build_nc(with_bias=with_bias)
    return _NC[with_bias]


def _prep_core_inputs(x, in_proj_w, in_proj_b, out_w, out_b):
    """Build the 8 per-core input dicts (host-side shard + transpose)."""
    import ml_dtypes
    bf16 = ml_dtypes.bfloat16
    fp8 = ml_dtypes.float8_e4m3
    # 0/1 keep-mask for S^T[k, q] diagonal blocks: keep where k <= q
    tri_np = (np.arange(128)[:, None] <= np.arange(128)[None, :])
    tri_bf = tri_np.astype(bf16)
    tri_f8 = tri_np.astype(fp8)
    id_bf = np.eye(128, dtype=np.float32).astype(bf16)
    ones_np = np.ones((1, 128), np.float32).astype(bf16)

    xT_by_b = [np.ascontiguousarray(np.asarray(x[b]).T) for b in range(B)]
    xT_bf = [a.astype(bf16) for a in xT_by_b]
    xT_f8 = [a.astype(fp8) for a in xT_by_b]

    in_maps = []
    for c in range(8):
        b = c // 2
        g = c % 2
        f0 = FPC * g
        Wq = np.asarray(in_proj_w[f0:f0 + FPC])
        Wk = np.asarray(in_proj_w[E + f0:E + f0 + FPC])
        Wv = np.asarray(in_proj_w[2 * E + f0:2 * E + f0 + FPC])
        bq = np.asarray(in_proj_b[f0:f0 + FPC])
        bk = np.asarray(in_proj_b[E + f0:E + f0 + FPC])
        bvv = np.asarray(in_proj_b[2 * E + f0:2 * E + f0 + FPC])
        Wo = np.asarray(out_w[:, f0:f0 + FPC])
        bqk_np = np.concatenate([bq, bk]).astype(np.float32).reshape(6, 128).T
        wqkT_np = np.ascontiguousarray(
            np.concatenate([Wq, Wk], axis=0).T).astype(np.float32)
        in_maps.append({
            "xT": xT_bf[b],
            "xT8": xT_f8[b],
            "wqkT": wqkT_np.astype(bf16),
            "wqkS": np.ascontiguousarray(np.concatenate(
                [wqkT_np[:, 0:128], wqkT_np[:, 384:512]],
                axis=1)).astype(bf16),
            "wqkT8": wqkT_np.astype(fp8),
            "wvT": np.ascontiguousarray(Wv.T).astype(bf16),
            "woT": np.ascontiguousarray(Wo.T).astype(bf16),
            "bqk": np.ascontiguousarray(bqk_np),
            "bv": bvv.reshape(1, FPC).astype(bf16),
            # out bias only on even cores so the host-side pair-sum is exact
            "bo": np.asarray(out_b).reshape(1, E).astype(bf16) if g == 0
                  else np.zeros((1, E), bf16),
            "tri": tri_bf,
            "tri8": tri_f8,
            "ident": id_bf,
            "ones": ones_np,
        })
    return in_maps


def kernel(x, in_proj_w, in_proj_b, out_w, out_b):
    zero_bias = (not np.any(np.asarray(in_proj_b))) and \
                (not np.any(np.asarray(out_b)))
    nc = _get_nc(with_bias=not zero_bias)
    in_maps = _prep_core_inputs(x, in_proj_w, in_proj_b, out_w, out_b)
    res = run_bass_kernel_spmd(nc, in_maps, core_ids=list(range(8)))
    out = np.empty((B, S, E), np.float32)
    for b in range(B):
        out[b] = res.results[2 * b]["y"] + res.results[2 * b + 1]["y"]
    return out


# revision 46
# speedup vs baseline: 1.0337x; 1.0027x over previous
"""Multi-head attention (B=4, S=2048, E=768, H=12, D=64, causal) on 8 trn2
NeuronCores.

Sharding: core c -> batch b = c//2, head-half g = c%2 (6 heads each).
Each core computes its 6 heads' attention plus the partial output
projection; the host sums the two half-head partials per batch.

Hybrid precision (validated against the fp32 reference in numpy):
  - Rows q < 512 (q-window 0) have concentrated softmax (few keys), so
    their whole attention path stays bf16: bf16 qk-projection (only the
    first 512 q/k columns are needed causally), bf16 scores, bf16 exp'd
    weights, bf16 ctx with bf16 V.
  - Rows q >= 512 average fp8 quantization noise over many keys: fp8e4
    q/k (from an fp8 DoubleRow qk-projection over e-pairs), plain-fp8
    score matmuls, exp'd weights written fp8e4 by ACT (bias -2.5 keeps
    them in fp8 range; softmax normalization cancels the shift), and the
    ctx matmul runs fp8 DoubleRow over k-chunk pairs (half the
    instructions, quarter the PE cycles of bf16).
  Combined max-rel-err ~8e-3 vs the 2e-2 gate.

On-device strategy (per core) otherwise matches the bf16 design:
  - Host pre-transposes x / weight slices so every contraction dim lands
    on SBUF partitions; x ships in bf16 and fp8.
  - Scores are computed TRANSPOSED (S^T[k, q] = K^T Q); diagonal tiles
    shrink their moving window; in-diagonal-block causal masking is a
    DVE multiply by a 0/1 lower-triangle.
  - V_aug packs a ones column per head so the PE emits softmax row-sums
    for free; V is stored bf16 (k-chunks 0-3, for the bf16 window) and
    fp8 in per-unit pair layout [128, 2, 65*HPC] for DoubleRow.
  - ctx lands [q, d+1] with q on partitions; normalization is a DVE
    reciprocal + per-qc scalar multiply; ONE merged [128,128] PE
    transpose per (hp, qw, qc) brings both heads' ctx^T back.
  - The attention phase is ACT(exp)-throughput-bound; projections and
    ctx/norm/out-proj run as deferred tasks interleaved between units.
"""
import sys, json, os

for _p in ("/opt/trn_rl_repo",):
    if _p not in sys.path and os.path.isdir(_p):
        sys.path.insert(0, _p)

import numpy as np
import concourse.bass as bass
import concourse.mybir as mybir
import concourse.tile as tile
from concourse.bass_utils import run_bass_kernel_spmd

B, S, E, H, D = 4, 2048, 768, 12, 64
HPC = H // 2          # heads per core = 6
FPC = HPC * D         # features per core per q/k/v = 384
EC = E // 128         # 6 contraction chunks for projections
SC = S // 128         # 16 s-chunks
QW = S // 512         # 4 q-windows
KC = S // 128         # 16 k-chunks
F32 = mybir.dt.float32
BF16 = mybir.dt.bfloat16
FP8 = mybir.dt.float8e4
EXP = mybir.ActivationFunctionType.Exp
DR = mybir.MatmulPerfMode.DoubleRow
EBIAS = -2.5          # exp bias for fp8 windows (softmax-invariant)


def _patch_multiwait(nc, max_waits=1):
    """This container's walrus rejects instructions with more than one sync
    wait. Split excess waits onto same-engine NOPs emitted immediately
    before the instruction (same-engine streams are order-preserving)."""
    raw = nc.to_json_bytes()
    m = json.loads(raw)
    for f in m["functions"]:
        for b in f["blocks"]:
            out = []
            for inst in b["instructions"]:
                si = inst.get("sync_info") or {}
                ws = si.get("on_wait") or []
                if len(ws) > max_waits:
                    eng = inst["engine"]
                    for i, w in enumerate(ws[:-max_waits]):
                        out.append({
                            "debug": inst.get("debug", 0), "engine": eng,
                            "ins": [], "name": inst["name"] + f"-mw{i}",
                            "opcode": "NoOp", "outs": [],
                            "sync_info": {"on_update": [], "on_wait": [w]},
                        })
                    si["on_wait"] = ws[-max_waits:]
                out.append(inst)
            b["instructions"] = out
    patched = json.dumps(m).encode()
    nc.to_json_bytes = lambda: patched
    return nc


def build_nc(with_bias=True):
    nc = bass.Bass()
    xT = nc.dram_tensor("xT", [E, S], BF16, kind="ExternalInput")
    xT8 = nc.dram_tensor("xT8", [E, S], FP8, kind="ExternalInput")
    wqkT = nc.dram_tensor("wqkT", [E, 2 * FPC], BF16, kind="ExternalInput")
    # startup slice: f-chunks 0 and 3 (cols 0:128 and 384:512) packed so the
    # two startup chains need ONE DMA (each DMA costs ~1.7us serial
    # HWDGE+transfer time at the head of the pipeline)
    wqkS = nc.dram_tensor("wqkS", [E, 256], BF16, kind="ExternalInput")
    wqkT8 = nc.dram_tensor("wqkT8", [E, 2 * FPC], FP8, kind="ExternalInput")
    wvT = nc.dram_tensor("wvT", [E, FPC], BF16, kind="ExternalInput")
    woT = nc.dram_tensor("woT", [FPC, E], BF16, kind="ExternalInput")
    bqk = nc.dram_tensor("bqk", [128, 2 * FPC // 128], F32, kind="ExternalInput")
    bv = nc.dram_tensor("bv", [1, FPC], BF16, kind="ExternalInput")
    bo = nc.dram_tensor("bo", [1, E], BF16, kind="ExternalInput")
    tri = nc.dram_tensor("tri", [128, 128], BF16, kind="ExternalInput")
    tri8 = nc.dram_tensor("tri8", [128, 128], FP8, kind="ExternalInput")
    ident = nc.dram_tensor("ident", [128, 128], BF16, kind="ExternalInput")
    ones = nc.dram_tensor("ones", [1, 128], BF16, kind="ExternalInput")
    y = nc.dram_tensor("y", [S, E], F32, kind="ExternalOutput")

    with tile.TileContext(nc) as tc, \
         nc.allow_low_precision(reason="hybrid bf16/fp8 pipeline by design"):
        with tc.tile_pool(name="persist", bufs=1) as P, \
             tc.tile_pool(name="ps", bufs=1, space="PSUM") as PS:
            # --- persistent tiles (bottom-of-stack, live whole kernel)
            # bf16 q/k: only columns [0, 512) are ever read (q-window 0)
            qkT_sb = [P.tile([128, 512], BF16, name=f"qkT{i}") for i in range(6)]
            # fp8 q/k: q chunks hold cols [512, 2048), k chunks [0, 2048)
            qkT8_sb = [P.tile([128, S], FP8, name=f"qkT8_{i}") for i in range(6)]
            # bf16 V for the bf16 window's ctx (k-chunks 0..3 only)
            V_sb = [P.tile([128, 65 * HPC], BF16, name=f"V{i}") for i in range(4)]
            # fp8 V in unit-pair layout: per partition [t(2), h(6), 65]
            V8_sb = [P.tile([128, 2 * 65 * HPC], FP8, name=f"V8_{i}")
                     for i in range(KC // 2)]
            ctxT_sb = [P.tile([128, S], BF16, name=f"ctxT{i}") for i in range(3)]
            woT_sb = [P.tile([128, E], BF16, name=f"woT{i}") for i in range(3)]
            bqk_sb = P.tile([128, 6], F32, name="bqk_sb")
            bv_sb = P.tile([1, FPC], BF16, name="bv_sb")
            bo_sb = P.tile([1, E], BF16, name="bo_sb")
            tri_sb = P.tile([128, 128], BF16, name="tri_sb")
            tri8_sb = P.tile([128, 128], FP8, name="tri8_sb")
            id_sb = P.tile([128, 128], BF16, name="id_sb")
            on_sb = P.tile([1, 128], BF16, name="on_sb")
            ebias_sb = P.tile([128, 1], F32, name="ebias_sb")
            nc.gpsimd.memset(ebias_sb[:], EBIAS)

            def ps_tile(shape, tag, bufs, dtype=F32):
                return PS.tile(shape, dtype, name=tag, tag=tag, bufs=bufs)

            # ============ phase 1 (projections) + attention, interleaved ====
            with tc.tile_pool(name="inp", bufs=1) as PI, \
                 tc.tile_pool(name="esb", bufs=14) as EP, \
                 tc.tile_pool(name="nrm", bufs=12) as NP, \
                 tc.tile_pool(name="osb", bufs=3) as OP:
                # consolidated input tiles; one DMA dispatch per tensor region
                xT_sb = PI.tile([128, EC * S], BF16, name="xT_all")
                xT8_sb = PI.tile([128, EC * S], FP8, name="xT8_all")
                wqkT_sb = PI.tile([128, EC * 2 * FPC], BF16, name="wqkT_all")
                wqkT8_sb = PI.tile([128, EC * 2 * FPC], FP8, name="wqkT8_all")
                wvT_sb = PI.tile([128, EC * FPC], BF16, name="wvT_all")
                wqkS_sb = PI.tile([128, EC * 256], BF16, name="wqkS_all")
                xs = xT_sb[:].rearrange("p (e s) -> p e s", e=EC)
                xd = xT.ap().rearrange("(e p) s -> p e s", p=128)
                x8s = xT8_sb[:].rearrange("p (e s) -> p e s", e=EC)
                x8d = xT8.ap().rearrange("(e p) s -> p e s", p=128)
                qs = wqkT_sb[:].rearrange("p (e f) -> p e f", e=EC)
                qd = wqkT.ap().rearrange("(e p) f -> p e f", p=128)
                q8s = wqkT8_sb[:].rearrange("p (e f) -> p e f", e=EC)
                q8d = wqkT8.ap().rearrange("(e p) f -> p e f", p=128)
                # DMA order: startup chains first (bf16 wqkT fo 0/3, then xT
                # cols 0:512 per e-chunk so the startup chains pipeline with
                # the DMA stream), then the rest in rough order of first use.
                nc.sync.dma_start(
                    wqkS_sb[:].rearrange("p (e f) -> p e f", e=EC),
                    wqkS.ap().rearrange("(e p) f -> p e f", p=128))
                nc.sync.dma_start(xs[:, 0:2, 0:512], xd[:, 0:2, 0:512])
                nc.sync.dma_start(xs[:, 2:6, 0:512], xd[:, 2:6, 0:512])
                nc.sync.dma_start(tri_sb[:], tri.ap())
                nc.sync.dma_start(q8s[:], q8d[:])
                nc.sync.dma_start(x8s[:, :, 0:1024], x8d[:, :, 0:1024])
                nc.sync.dma_start(
                    wvT_sb[:].rearrange("p (e f) -> p e f", e=EC),
                    wvT.ap().rearrange("(e p) f -> p e f", p=128))
                nc.sync.dma_start(tri8_sb[:], tri8.ap())
                nc.sync.dma_start(xs[:, :, 512:S], xd[:, :, 512:S])
                nc.sync.dma_start(x8s[:, :, 1024:S], x8d[:, :, 1024:S])
                nc.sync.dma_start(qs[:, :, 128:384], qd[:, :, 128:384])
                nc.sync.dma_start(qs[:, :, 512:768], qd[:, :, 512:768])
                nc.sync.dma_start(id_sb[:], ident.ap())
                for i in range(3):
                    nc.sync.dma_start(woT_sb[i][:],
                                      woT.ap()[128 * i:128 * (i + 1), :])
                nc.sync.dma_start(bqk_sb[:], bqk.ap())
                nc.sync.dma_start(bv_sb[:], bv.ap())
                nc.sync.dma_start(on_sb[:], ones.ap())
                nc.sync.dma_start(bo_sb[:], bo.ap())

                def emit_qk_chain(fo, on_act=False, cols=(0, 512)):
                    """bf16 qk-proj for f-chunk fo, s-cols [0, 512) only
                    (optionally a sub-range, for the startup k-chain).
                    f-chunks 0/3 read the packed one-DMA startup tile."""
                    c0, c1 = cols
                    pair = ps_tile([128, 512], "ppro_t", 2)
                    for ecc in range(EC):
                        if fo in (0, 3):
                            stat = wqkS_sb[:, 256 * ecc + 128 * (fo // 3):
                                           256 * ecc + 128 * (fo // 3 + 1)]
                        else:
                            stat = wqkT_sb[:, 768 * ecc + 128 * fo:
                                           768 * ecc + 128 * (fo + 1)]
                        nc.tensor.matmul(
                            pair[:, c0:c1],
                            stat,
                            xT_sb[:, S * ecc + c0:S * ecc + c1],
                            start=(ecc == 0), stop=(ecc == EC - 1),
                            skip_group_check=True)
                    dst = qkT_sb[fo][:, c0:c1]
                    src = pair[:, c0:c1]
                    if with_bias:
                        nc.vector.tensor_scalar_add(
                            dst, src, bqk_sb[:, fo:fo + 1])
                    elif on_act:
                        nc.scalar.copy(dst, src)
                    else:
                        nc.vector.tensor_copy(dst, src)

                def emit_qk8_chain(fo, win):
                    """fp8 DoubleRow qk-proj for f-chunk fo, one 256-col
                    window. q chunks (fo<3): win 0..5 -> cols 512+256*win.
                    k chunks (fo>=3): win 0..7 -> cols 256*win."""
                    c0 = (512 if fo < 3 else 0) + 256 * win
                    pair = ps_tile([128, 512], "ppro_t", 2)
                    wv = wqkT8_sb[:].rearrange("p (e f) -> p e f", e=EC)
                    xv = xT8_sb[:].rearrange("p (e s) -> p e s", e=EC)
                    for ecp in range(EC // 2):
                        nc.tensor.matmul(
                            pair[:, 0:256],
                            wv[:, 2 * ecp:2 * ecp + 2,
                               128 * fo:128 * (fo + 1)],
                            xv[:, 2 * ecp:2 * ecp + 2, c0:c0 + 256],
                            start=(ecp == 0), stop=(ecp == EC // 2 - 1),
                            skip_group_check=True, perf_mode=DR)
                    dst = qkT8_sb[fo][:, c0:c0 + 256]
                    src = pair[:, 0:256]
                    if with_bias:
                        nc.vector.tensor_scalar_add(
                            dst, src, bqk_sb[:, fo:fo + 1])
                    else:
                        nc.vector.tensor_copy(dst, src)

                def emit_v_chunk(sc):
                    """V-proj for s-chunk sc (bf16), copied out bf16 (sc<4,
                    for the bf16 window) and fp8 (unit-pair layout, always)."""
                    psv = ps_tile([128, FPC], "ppro_t", 2)
                    for ecc in range(EC):
                        nc.tensor.matmul(
                            psv[:],
                            xT_sb[:, S * ecc + 128 * sc:
                                  S * ecc + 128 * (sc + 1)],
                            wvT_sb[:, FPC * ecc:FPC * (ecc + 1)],
                            start=(ecc == 0),
                            stop=(not with_bias and ecc == EC - 1),
                            skip_group_check=True)
                    if with_bias:
                        nc.tensor.matmul(psv[:], on_sb[:, 0:128],
                                         bv_sb[:], start=False, stop=True,
                                         skip_group_check=True)
                    u, t = divmod(sc, 2)
                    v8 = V8_sb[u][:].rearrange("p (t h x) -> p t h x",
                                               t=2, x=65)
                    nc.vector.tensor_copy(
                        v8[:, t, :, 0:64],
                        psv[:].rearrange("p (h x) -> p h x", x=64))
                    nc.gpsimd.memset(v8[:, t, :, 64:65], 1.0)
                    if sc < 4:
                        vv = V_sb[sc][:].rearrange("p (h x) -> p h x", x=65)
                        nc.vector.tensor_copy(
                            vv[:, :, 0:64],
                            psv[:].rearrange("p (h x) -> p h x", x=64))
                        nc.gpsimd.memset(vv[:, :, 64:65], 1.0)

                def emit_scores(hp, qw, u):
                    """Scores S^T[k, q] for a pair of k-chunks, both heads,
                    + exp (+ DVE causal masks on diag blocks). qw=0 runs
                    bf16 (writes Et bf16); qw>=1 runs fp8 (writes Et fp8,
                    exp bias EBIAS). Returns Et [128, 2048]
                    (cols 1024*hd + 512*half + qlocal)."""
                    fp8 = qw >= 1
                    if fp8:
                        qT, kT = qkT8_sb[hp], qkT8_sb[3 + hp]
                        qcol = 512 * qw
                        trit = tri8_sb
                        edt = FP8
                    else:
                        qT, kT = qkT_sb[hp], qkT_sb[3 + hp]
                        qcol = 0       # bf16 tiles hold only cols [0,512)
                        trit = tri_sb
                        edt = BF16
                    pss = {hd: ps_tile([128, 1024], "pss_t", 2)
                           for hd in range(2)}
                    Et = EP.tile([128, 2048], edt, name="E_t")
                    for half in range(2):
                        ki = 2 * u + half
                        j = ki - 4 * qw
                        c = 128 * j if j > 0 else 0
                        # strict row-group alternation (base 0,64,0,64) so
                        # score matmul pairs run on separate PE row groups;
                        # diag tiles shrink the moving window
                        for hd in range(2):
                            base = 64 * hd
                            nc.tensor.matmul(
                                pss[hd][:, 512 * half + c:512 * (half + 1)],
                                kT[base:base + 64, 128 * ki:128 * (ki + 1)],
                                qT[base:base + 64,
                                   qcol + c:qcol + 512],
                                start=True, stop=True,
                                skip_group_check=True)
                    j0 = 2 * u - 4 * qw
                    j1 = j0 + 1
                    c0 = 128 * j0 if j0 > 0 else 0
                    bias = ebias_sb[:, 0:1] if fp8 else 0.0
                    # one exp per head spans both halves when contiguous;
                    # when the half-1 diag shrink leaves an unwritten PSUM
                    # gap, split the exp around it -- EXCEPT in fp8 windows
                    # with a single-block gap (j1 == 1): there one merged exp
                    # over the gap is cheaper than a second instruction. The
                    # gap columns hold stale-but-bounded old scores (the slot
                    # was written by earlier units), their exp is finite, and
                    # the masked-out block is never read by any ctx matmul.
                    for hd in range(2):
                        if j1 == 1 and (fp8 or hp > 0):
                            nc.scalar.activation(
                                Et[:, 1024 * hd:1024 * (hd + 1)],
                                pss[hd][:, 0:1024], EXP, scale=0.125,
                                bias=bias)
                        elif j1 > 0:
                            nc.scalar.activation(
                                Et[:, 1024 * hd + c0:1024 * hd + 512],
                                pss[hd][:, c0:512], EXP, scale=0.125,
                                bias=bias)
                            c1 = 128 * j1
                            nc.scalar.activation(
                                Et[:, 1024 * hd + 512 + c1:1024 * (hd + 1)],
                                pss[hd][:, 512 + c1:1024], EXP, scale=0.125,
                                bias=bias)
                        else:
                            nc.scalar.activation(
                                Et[:, 1024 * hd + c0:1024 * (hd + 1)],
                                pss[hd][:, c0:1024], EXP, scale=0.125,
                                bias=bias)
                    # causal mask inside the diagonal 128x128 blocks
                    for half in range(2):
                        j = 2 * u + half - 4 * qw
                        if j >= 0:
                            for hd in range(2):
                                off = 1024 * hd + 512 * half + 128 * j
                                nc.vector.tensor_mul(
                                    Et[:, off:off + 128],
                                    Et[:, off:off + 128], trit[:])
                    return Et

                def emit_ctx_qc(hp, qw, qc, Ets, psc):
                    """ctx for one q-chunk, both heads. qw=0: bf16 per-ki
                    matmuls with V_sb. qw>=1: fp8 DoubleRow over k-chunk
                    pairs (units) with V8_sb, plus a plain-fp8 tail when
                    the diagonal cuts a unit in half."""
                    nk = 4 * qw + qc + 1        # k-chunks 0..4qw+qc
                    if qw == 0:
                        for ki in range(nk):
                            u, half = divmod(ki, 2)
                            Et = Ets[u]
                            for hd in range(2):
                                h = 2 * hp + hd
                                nc.tensor.matmul(
                                    psc[hd][:, 65 * qc:65 * qc + 65],
                                    Et[:, 1024 * hd + 512 * half + 128 * qc:
                                        1024 * hd + 512 * half + 128 * qc
                                        + 128],
                                    V_sb[ki][:, 65 * h:65 * h + 65],
                                    start=(ki == 0), stop=(ki == nk - 1),
                                    skip_group_check=True)
                        return
                    nu_full = nk // 2           # full DR unit-pairs
                    tail = nk % 2               # lone half-0 chunk at the end
                    for hd in range(2):
                        h = 2 * hp + hd
                        for u in range(nu_full):
                            ev = Ets[u][:, 1024 * hd:1024 * (hd + 1)] \
                                .rearrange("p (t q) -> p t q", t=2)
                            v8 = V8_sb[u][:].rearrange(
                                "p (t h x) -> p t h x", t=2, x=65)
                            nc.tensor.matmul(
                                psc[hd][:, 65 * qc:65 * qc + 65],
                                ev[:, :, 128 * qc:128 * qc + 128],
                                v8[:, :, h, :],
                                start=(u == 0),
                                stop=(tail == 0 and u == nu_full - 1),
                                skip_group_check=True, perf_mode=DR)
                        if tail:
                            u = nu_full
                            v8 = V8_sb[u][:].rearrange(
                                "p (t h x) -> p t h x", t=2, x=65)
                            nc.tensor.matmul(
                                psc[hd][:, 65 * qc:65 * qc + 65],
                                Ets[u][:, 1024 * hd + 128 * qc:
                                       1024 * hd + 128 * qc + 128],
                                v8[:, 0, h, :],
                                start=(nu_full == 0), stop=True,
                                skip_group_check=True)

                def emit_finish_qc(hp, qw, qc, psc, st, outproj=True):
                    """Per-qc finish for the final step: craw slice + recip +
                    normalize + transpose + ctxT copy + out-proj, so each
                    s-chunk's chain starts the moment its own ctx is done."""
                    if "ctxn" not in st:
                        st["ctxn"] = NP.tile([128, 512], BF16, name="ctxn_t")
                    # psc_t's 2 slots are BOTH live (the psc accumulators)
                    # until the last craw is copied out, so pt must come from
                    # the pss_t ring (scores are done with it by now).
                    pt = ps_tile([128, 512], "pss_t", 2, dtype=BF16)
                    ctxn = st["ctxn"]
                    for hd in range(2):
                        craw = NP.tile([128, 65], F32, name="crawq_t")
                        nc.vector.tensor_copy(
                            craw[:], psc[hd][:, 65 * qc:65 * qc + 65])
                        rinv = NP.tile([128, 1], F32, name="rinvq_t")
                        nc.vector.reciprocal(
                            rinv[:], craw[:, 64:65])
                        nc.vector.tensor_scalar_mul(
                            ctxn[:, 128 * qc + 64 * hd:
                                 128 * qc + 64 * (hd + 1)],
                            craw[:, 0:64], rinv[:])
                    nc.tensor.transpose(
                        pt[:, 128 * qc:128 * (qc + 1)],
                        ctxn[:, 128 * qc:128 * (qc + 1)], id_sb[:])
                    nc.scalar.copy(
                        ctxT_sb[hp][:, 512 * qw + 128 * qc:
                                     512 * qw + 128 * (qc + 1)],
                        pt[:, 128 * qc:128 * (qc + 1)])
                    if outproj:
                        emit_outproj_sc(4 * qw + qc, tail=True)

                def emit_norm_head(hp, qw, psc, st):
                    """Copy raw ctx out of PSUM (freeing psc) and compute
                    reciprocal row-sums."""
                    craws = []
                    for hd in range(2):
                        craw = NP.tile([128, 260], F32, name="craw_t")
                        nc.vector.tensor_copy(craw[:], psc[hd][:])
                        craws.append(craw)
                    pt = ps_tile([128, 512], "psc_t", 2, dtype=BF16)
                    ctxn = NP.tile([128, 512], BF16, name="ctxn_t")
                    rinvs = []
                    for hd in range(2):
                        craw = craws[hd]
                        cv = craw[:].rearrange("p (q x) -> p q x", x=65)
                        rinv = NP.tile([128, 4], F32, name="rinv_t")
                        nc.vector.reciprocal(
                            rinv[:].rearrange("p (q x) -> p q x", x=1),
                            cv[:, :, 64:65])
                        rinvs.append((craw, rinv))
                    st["pt"] = pt
                    st["ctxn"] = ctxn
                    st["rinvs"] = rinvs

                def emit_norm_qc(hp, qw, qc, st):
                    """Normalize + ONE merged 2-head transpose + copy out one
                    128-column ctxT block."""
                    pt, ctxn, rinvs = st["pt"], st["ctxn"], st["rinvs"]
                    for hd in range(2):
                        craw, rinv = rinvs[hd]
                        nc.vector.tensor_scalar_mul(
                            ctxn[:, 128 * qc + 64 * hd:
                                 128 * qc + 64 * (hd + 1)],
                            craw[:, 65 * qc:65 * qc + 64],
                            rinv[:, qc:qc + 1])
                    nc.tensor.transpose(
                        pt[:, 128 * qc:128 * (qc + 1)],
                        ctxn[:, 128 * qc:128 * (qc + 1)], id_sb[:])
                    nc.vector.tensor_copy(
                        ctxT_sb[hp][:, 512 * qw + 128 * qc:
                                     512 * qw + 128 * (qc + 1)],
                        pt[:, 128 * qc:128 * (qc + 1)])

                def emit_outproj_sc(sc, tail=False, ring=None):
                    osb = OP.tile([128, E], F32, name="osb_t")
                    # tail chains run while psc_t's two slots are still live
                    # (the psc accumulators), so they must use pss_t;
                    # explicit ring= alternates banks for back-to-back chains
                    tg, nb = (ring, 2) if ring else \
                        (("pss_t", 2) if tail else ("psc_t", 2))
                    pos = {0: ps_tile([128, 512], tg, nb),
                           512: ps_tile([128, 256], tg, nb)}
                    for c in range(3):
                        for f0, fn in ((0, 512), (512, 256)):
                            nc.tensor.matmul(
                                pos[f0][:, 0:fn],
                                ctxT_sb[c][:, 128 * sc:128 * (sc + 1)],
                                woT_sb[c][:, f0:f0 + fn],
                                start=(c == 0),
                                stop=(not with_bias and c == 2),
                                skip_group_check=True)
                    for f0, fn in ((0, 512), (512, 256)):
                        if with_bias:
                            nc.tensor.matmul(pos[f0][:, 0:fn],
                                             on_sb[:, 0:128],
                                             bo_sb[:, f0:f0 + fn],
                                             start=False, stop=True,
                                             skip_group_check=True)
                        # in the tail ACT is idle (all exps done): put the
                        # copy-outs there and DMA each f-window as it lands
                        if tail:
                            nc.scalar.copy(osb[:, f0:f0 + fn],
                                           pos[f0][:, 0:fn])
                            nc.sync.dma_start(
                                y.ap()[128 * sc:128 * (sc + 1), f0:f0 + fn],
                                osb[:, f0:f0 + fn])
                        else:
                            nc.vector.tensor_copy(osb[:, f0:f0 + fn],
                                                  pos[f0][:, 0:fn])
                    if not tail:
                        nc.sync.dma_start(y.ap()[128 * sc:128 * (sc + 1), :],
                                          osb[:])

                def emit_attention():
                    # software pipeline: ctx runs as per-q-chunk deferred
                    # tasks queued when a step's scores complete; one task
                    # pops per unit so ctx/norm/out-proj spread between
                    # later units while ACT chews on exps.
                    work = []

                    def flush_one():
                        if work:
                            work.pop(0)()
                        if len(work) > 3:
                            work.pop(0)()

                    def make_step(hp, qw, Ets):
                        holder = {}

                        def get_psc():
                            if not holder:
                                holder[0] = {
                                    hd: ps_tile([128, 260], "psc_t", 2)
                                    for hd in range(2)}
                            return holder[0]

                        def ctx_task(qc):
                            return lambda: emit_ctx_qc(
                                hp, qw, qc, Ets, get_psc())
                        return get_psc, ctx_task

                    def finish_step(hp, qw, get_psc):
                        st = {}
                        if hp < 2:
                            def norm_all():
                                emit_norm_head(hp, qw, get_psc(), st)
                                for qc in range(4):
                                    emit_norm_qc(hp, qw, qc, st)
                            work.append(norm_all)
                        else:
                            work.append(lambda: emit_norm_head(
                                hp, qw, get_psc(), st))

                            def norm_op(qc):
                                emit_norm_qc(hp, qw, qc, st)
                                emit_outproj_sc(4 * qw + qc)
                            for qc in range(4):
                                work.append(lambda qc=qc: norm_op(qc))

                    # phase-1 chains interleaved between attention units:
                    # (hp, qw, u) -> thunks emitted right after that unit's
                    # scores+flush. Deadlines:
                    #   bf16 qk chain fo in {hp', 3+hp'} before (hp', 0, 0)
                    #   fp8 k chain (3+hp', win w) before (hp', 1 + w//4,
                    #     u = w % ...) - k-chunks 2w,2w+1 first read at
                    #     (hp', qw >= 1, u = w)
                    #   fp8 q chain (hp', win w) before (hp', 1 + w//2, 0)
                    #   V chunk sc: read by ctx tasks popping ~one step later
                    intra = {}

                    def add(hp, qw, u, fn):
                        intra.setdefault((hp, qw, u), []).append(fn)

                    def addv(hp, qw, u, sc):
                        add(hp, qw, u, lambda: emit_v_chunk(sc))

                    def addqk(hp, qw, u, fo):
                        add(hp, qw, u, lambda: emit_qk_chain(fo))

                    def addq8(hp, qw, u, fo, w):
                        add(hp, qw, u, lambda: emit_qk8_chain(fo, w))

                    # --- hp 0 --- (all 16 V chunks live here; k8(3,*) and
                    # q8(0,*) feed hp0's own fp8 windows)
                    # Step order: plain nested (hp, qw); (2,3) runs
                    # last with the inline early-ctx finish.
                    steps = [(hp, qw) for hp in range(3)
                             for qw in range(QW)]
                    # Placement rule: an intra item at slot (hp, qw, u) is
                    # emitted AFTER scores of unit u+1 (emit-ahead pipeline),
                    # so it may only feed units >= u+2 of its own step.
                    addq8(0, 0, 0, 3, 0)
                    addq8(0, 0, 0, 0, 0)
                    addv(0, 0, 0, 0)
                    addq8(0, 0, 1, 3, 1)
                    addq8(0, 0, 1, 0, 1)
                    addv(0, 0, 1, 1)
                    addv(0, 1, 0, 2)
                    addq8(0, 1, 0, 3, 2)
                    addv(0, 1, 1, 3)
                    addq8(0, 1, 1, 3, 3)
                    addq8(0, 1, 1, 0, 2)
                    addq8(0, 1, 2, 0, 3)
                    addv(0, 1, 3, 4)
                    addq8(0, 1, 3, 3, 4)
                    addv(0, 2, 0, 5)
                    addq8(0, 2, 0, 3, 5)
                    addv(0, 2, 1, 6)
                    addq8(0, 2, 1, 0, 4)
                    addv(0, 2, 2, 7)
                    addq8(0, 2, 3, 0, 5)
                    addv(0, 2, 3, 8)
                    addq8(0, 2, 4, 3, 6)
                    addv(0, 2, 4, 9)
                    addq8(0, 2, 5, 3, 7)
                    addv(0, 3, 0, 10)
                    addv(0, 3, 1, 11)
                    addv(0, 3, 2, 12)
                    addv(0, 3, 3, 13)
                    addv(0, 3, 4, 14)
                    addqk(0, 3, 5, 1)
                    addv(0, 3, 5, 15)
                    addqk(0, 3, 6, 4)
                    addq8(0, 3, 6, 4, 0)
                    addq8(0, 3, 7, 1, 0)
                    addq8(1, 0, 0, 4, 1)
                    addq8(1, 0, 0, 1, 1)
                    addq8(1, 0, 1, 4, 2)
                    addq8(1, 1, 0, 4, 3)
                    addq8(1, 1, 1, 1, 2)
                    addq8(1, 1, 2, 1, 3)
                    addq8(1, 1, 3, 4, 4)
                    addq8(1, 2, 0, 4, 5)
                    addq8(1, 2, 1, 1, 4)
                    addq8(1, 2, 3, 1, 5)
                    addq8(1, 2, 4, 4, 6)
                    addq8(1, 2, 5, 4, 7)
                    addqk(1, 3, 1, 2)
                    addqk(1, 3, 2, 5)
                    addq8(1, 3, 3, 5, 0)
                    addq8(1, 3, 4, 5, 1)
                    addq8(1, 3, 5, 2, 0)
                    addq8(1, 3, 6, 2, 1)
                    addq8(1, 3, 7, 5, 2)
                    addq8(2, 0, 0, 5, 3)
                    addq8(2, 0, 1, 2, 2)
                    addq8(2, 1, 0, 5, 4)
                    addq8(2, 1, 1, 2, 3)
                    addq8(2, 1, 2, 5, 5)
                    addq8(2, 2, 0, 5, 6)
                    addq8(2, 2, 1, 2, 4)
                    addq8(2, 2, 3, 2, 5)
                    addq8(2, 2, 4, 5, 7)

                    for si, (hp, qw) in enumerate(steps):
                        nu = 2 * qw + 2
                        last = (si == len(steps) - 1)
                        Ets = []
                        get_psc, ctx_task = make_step(hp, qw, Ets)
                        if last:
                            # run all but the final two units normally (with
                            # harder draining), then emit the last two units'
                            # scores up front so ACT stays fed while the PE
                            # drains the backlog and runs the tail chains
                            Ets.append(emit_scores(hp, qw, 0))
                            for u in range(nu - 2):
                                if u + 1 < nu - 2:
                                    Ets.append(emit_scores(hp, qw, u + 1))
                                for fn in intra.get((hp, qw, u), ()):
                                    fn()
                                flush_one()
                            Ets.append(emit_scores(hp, qw, nu - 2))
                            Ets.append(emit_scores(hp, qw, nu - 1))
                            while work:
                                work.pop(0)()
                            st = {}
                            for qc in range(4):
                                ctx_task(qc)()
                            for qc in range(4):
                                emit_finish_qc(hp, qw, qc, get_psc(), st,
                                               outproj=False)
                            for qc in range(4):
                                emit_outproj_sc(4 * qw + qc, tail=True)
                            continue
                        # emit-ahead software pipeline                        # emit-ahead software pipeline: unit u+1's scores go
                        # out BEFORE unit u's intra chains / deferred pops, so
                        # the next exps are never queued behind filler work on
                        # the in-order PE stream.
                        Ets.append(emit_scores(hp, qw, 0))
                        for u in range(nu):
                            if u + 1 < nu:
                                Ets.append(emit_scores(hp, qw, u + 1))
                            for fn in intra.get((hp, qw, u), ()):
                                fn()
                            flush_one()
                        for qc in range(4):
                            work.append(ctx_task(qc))
                        finish_step(hp, qw, get_psc)
                    while work:
                        flush_one()

                # PE p-state warmup: dummy matmuls on a memset tile keep the
                # PE busy through the startup DMAs so the real startup chains
                # run at a higher clock (the cost model ramps 0.65 -> 1.2 ->
                # 2.4 GHz with continuous execution).
                warm_sb = PI.tile([128, 128], BF16, name="warm_sb")
                nc.gpsimd.memset(warm_sb[:], 0.5)
                wps = ps_tile([128, 512], "ppro_t", 2)
                for _ in range(34):
                    nc.tensor.matmul(wps[:, 0:128], warm_sb[:], warm_sb[:],
                                     start=True, stop=True,
                                     skip_group_check=True)

                # start-up: only what the first scores need
                emit_qk_chain(0)
                emit_qk_chain(3, on_act=True)
                emit_attention()

    return _patch_multiwait(nc)


_NC = {}


def _get_nc(with_bias=True):
    if with_bias not in _NC:
        _NC[with_bias] = build_nc(with_bias=with_bias)
    return _NC[with_bias]


def _prep_core_inputs(x, in_proj_w, in_proj_b, out_w, out_b):
    """Build the 8 per-core input dicts (host-side shard + transpose)."""
    import ml_dtypes
    bf16 = ml_dtypes.bfloat16
    fp8 = ml_dtypes.float8_e4m3
    # 0/1 keep-mask for S^T[k, q] diagonal blocks: keep where k <= q
    tri_np = (np.arange(128)[:, None] <= np.arange(128)[None, :])
    tri_bf = tri_np.astype(bf16)
    tri_f8 = tri_np.astype(fp8)
    id_bf = np.eye(128, dtype=np.float32).astype(bf16)
    ones_np = np.ones((1, 128), np.float32).astype(bf16)

    xT_by_b = [np.ascontiguousarray(np.asarray(x[b]).T) for b in range(B)]
    xT_bf = [a.astype(bf16) for a in xT_by_b]
    xT_f8 = [a.astype(fp8) for a in xT_by_b]

    in_maps = []
    for c in range(8):
        b = c // 2
        g = c % 2
        f0 = FPC * g
        Wq = np.asarray(in_proj_w[f0:f0 + FPC])
        Wk = np.asarray(in_proj_w[E + f0:E + f0 + FPC])
        Wv = np.asarray(in_proj_w[2 * E + f0:2 * E + f0 + FPC])
        bq = np.asarray(in_proj_b[f0:f0 + FPC])
        bk = np.asarray(in_proj_b[E + f0:E + f0 + FPC])
        bvv = np.asarray(in_proj_b[2 * E + f0:2 * E + f0 + FPC])
        Wo = np.asarray(out_w[:, f0:f0 + FPC])
        bqk_np = np.concatenate([bq, bk]).astype(np.float32).reshape(6, 128).T
        wqkT_np = np.ascontiguousarray(
            np.concatenate([Wq, Wk], axis=0).T).astype(np.float32)
        in_maps.append({
            "xT": xT_bf[b],
            "xT8": xT_f8[b],
            "wqkT": wqkT_np.astype(bf16),
            "wqkS": np.ascontiguousarray(np.concatenate(
                [wqkT_np[:, 0:128], wqkT_np[:, 384:512]],
                axis=1)).astype(bf16),
            "wqkT8": wqkT_np.astype(fp8),
            "wvT": np.ascontiguousarray(Wv.T).astype(bf16),
            "woT": np.ascontiguousarray(Wo.T).astype(bf16),
            "bqk": np.ascontiguousarray(bqk_np),
            "bv": bvv.reshape(1, FPC).astype(bf16),
            # out bias only on even cores so the host-side pair-sum is exact
            "bo": np.asarray(out_b).reshape(1, E).astype(bf16) if g == 0
                  else np.zeros((1, E), bf16),
            "tri": tri_bf,
            "tri8": tri_f8,
            "ident": id_bf,
            "ones": ones_np,
        })
    return in_maps


def kernel(x, in_proj_w, in_proj_b, out_w, out_b):
    zero_bias = (not np.any(np.asarray(in_proj_b))) and \
                (not np.any(np.asarray(out_b)))
    nc = _get_nc(with_bias=not zero_bias)
    in_maps = _prep_core_inputs(x, in_proj_w, in_proj_b, out_w, out_b)
    res = run_bass_kernel_spmd(nc, in_maps, core_ids=list(range(8)))
    out = np.empty((B, S, E), np.float32)
    for b in range(B):
        out[b] = res.results[2 * b]["y"] + res.results[2 * b + 1]["y"]
    return out
